# revision 8
# baseline (speedup 1.0000x reference)
"""Trainium2 Bass kernel v2 for nn_CamAttnCon (topk-masked CAM attention).

Strategy (per core, 4 samples, D-layout fp16):
  - emb shipped host-transposed [BL, D, T] fp16; one big DMA per sample.
  - num[t] = e.f via ap-1 PE matmuls (lhsT = emb chunk [128D,128T], rhs = fore
    col) -> out [128,1] PSUM, nearly free in the cost model.
  - xn2[t] via tensor_scalar pow(e,2) (one DVE op per sample) + ap-1 ones
    matmuls.
  - w = num * rsqrt(xn2), masked; fp16 w16.  Selection threshold theta found
    by a 2-round 128-candidate parallel count search on a broadcast w (exact
    because round-2 step < fp16 ulp at theta).
  - Compaction by prefix-sum of the selection mask (triangular + ones PE
    matmuls), tie-robust; one-hot is_equal -> compact (row index, gather
    weight) pairs via PE; indirect-DMA gather of selected fp16 attn rows.
  - tot[s] = sum_j g_j att[t_j, h, s] over h via ap-1 matmuls in s-partition
    space; transpose back; min-max normalize.
"""

import os
import sys

sys.path.insert(0, "/opt/trn_rl_repo")

import numpy as np
from contextlib import ExitStack

import concourse.bass as bass
import concourse.bacc as bacc
import concourse.mybir as mybir
import concourse.tile as tile
from concourse.masks import make_identity
from concourse import bass_utils

f32 = mybir.dt.float32
fp16 = mybir.dt.float16
i32 = mybir.dt.int32
AX = mybir.AxisListType
OP = mybir.AluOpType
AF = mybir.ActivationFunctionType

B, T, D, H, S = 32, 512, 512, 8, 196
NCORES = 8
BL = B // NCORES            # 4 samples per core
TC = T // 128               # 4 t-chunks of 128
DC = D // 128               # 4 d-chunks of 128
HS = H * S                  # 1568
KK = 51                     # max top-k count
J = 52                      # padded slot count (>= KK, slots 1..52 hold pos)
NB = BL * TC                # 16 (b,tc) columns

# threshold search grid: theta in [LO0, LO0 + 128*STEP1)
LO0 = 0.0
STEP1 = 4.0 / 128.0         # 0.03125
STEP2 = STEP1 / 128.0       # 2.44e-4 < fp16 ulp at theta (~9.8e-4)

LAST_EXEC_NS = None
LAST_RESULTS = None


def build_body(ctx, tc_, emb, att, fore, tgtT, out):
    nc = tc_.nc

    const = ctx.enter_context(tc_.tile_pool(name="const", bufs=1))
    small = ctx.enter_context(tc_.tile_pool(name="small", bufs=1))
    embp = ctx.enter_context(tc_.tile_pool(name="embp", bufs=4))
    sqp = ctx.enter_context(tc_.tile_pool(name="sqp", bufs=2))
    wbcp = ctx.enter_context(tc_.tile_pool(name="wbcp", bufs=4))
    gatp = ctx.enter_context(tc_.tile_pool(name="gatp", bufs=4))

    ps_nx = ctx.enter_context(tc_.tile_pool(name="ps_nx", bufs=1, space="PSUM"))
    ps_wbc = ctx.enter_context(tc_.tile_pool(name="ps_wbc", bufs=1, space="PSUM"))
    ps_sm = ctx.enter_context(tc_.tile_pool(name="ps_sm", bufs=2, space="PSUM"))
    ps_tot = ctx.enter_context(tc_.tile_pool(name="ps_tot", bufs=1, space="PSUM"))

    # ---------------- input DMAs (tgt/fore first: tiny; then emb) -----------
    tgt_c = small.tile([128, NB], i32, tag="tgt_c")
    nc.sync.dma_start(tgt_c[:], tgtT[:])
    fore_c = small.tile([128, BL * DC], fp16, tag="fore_c")
    nc.scalar.dma_start(fore_c[:], fore[:])
    embR = emb.rearrange("b (dc p) t -> b p dc t", p=128)
    embt = []
    for b in range(BL):
        e = embp.tile([128, DC * T], fp16, tag="emb")
        nc.sync.dma_start(e[:].rearrange("p (dc t) -> p dc t", dc=DC), embR[b])
        embt.append(e)

    # ---------------- constants (Pool/ACT, overlap the DMAs) ----------------
    # warm ACT function tables used later (rsqrt / reciprocal / copy)
    warm = const.tile([1, 1], f32, tag="warm")
    nc.vector.memset(warm[:], 1.0)
    warm2 = const.tile([1, 1], f32, tag="warm2")
    nc.scalar.sqrt(warm2[:], warm[:])
    nc.scalar.copy(warm2[:], warm[:])

    ones_col = const.tile([128, 1], fp16, tag="ones_col")
    nc.gpsimd.memset(ones_col[:], 1.0)
    ones_row = const.tile([1, 128], fp16, tag="ones_row")
    nc.gpsimd.memset(ones_row[:], 1.0)
    ones_row32 = const.tile([1, 128], f32, tag="ones_row32")
    nc.gpsimd.memset(ones_row32[:], 1.0)
    ones128 = const.tile([128, 128], fp16, tag="ones128")
    nc.gpsimd.memset(ones128[:], 1.0)
    zero_col = const.tile([128, 1], fp16, tag="zero_col")
    nc.gpsimd.memset(zero_col[:], 0.0)
    # bcsel4: slice tc [4, 128] has row tc all-ones (partition-bcast selector)
    bcsel4 = const.tile([BL, TC * 128], fp16, tag="bcsel4")
    nc.gpsimd.memset(bcsel4[:], 0.0)
    nc.gpsimd.affine_select(
        out=bcsel4[:].rearrange("p (blk j) -> p blk j", blk=TC),
        in_=bcsel4[:].rearrange("p (blk j) -> p blk j", blk=TC),
        compare_op=OP.not_equal,
        fill=1.0,
        base=0,
        pattern=[[-1, TC], [0, 128]],
        channel_multiplier=1,
    )

    # TRI[p, q] = 1 if p <= q  (inclusive prefix along partitions)
    qv_i = const.tile([128, 128], i32, tag="qv_i")
    nc.gpsimd.iota(qv_i[:], pattern=[[1, 128]], base=0, channel_multiplier=0)
    qv = const.tile([128, 128], f32, tag="qv")
    nc.gpsimd.tensor_copy(qv[:], qv_i[:])

    id16 = const.tile([128, 128], fp16, tag="id16")
    nc.gpsimd.memset(id16[:], 0.0)
    nc.gpsimd.affine_select(
        out=id16[:], in_=id16[:], compare_op=OP.not_equal, fill=1.0,
        base=0, pattern=[[1, 128]], channel_multiplier=-1,
    )
    id4 = const.tile([4, 4], f32, tag="id4")
    make_identity(nc, id4[:])

    # jvrep[p, (j, c)] = j + 1   (slot match values, c-packed for 2x mode)
    jvrep_i = const.tile([128, J * TC], i32, tag="jvrep_i")
    nc.gpsimd.iota(jvrep_i[:], pattern=[[1, J], [0, TC]], base=1, channel_multiplier=0)
    jvrep = const.tile([128, J * TC], fp16, tag="jvrep")
    nc.gpsimd.tensor_copy(jvrep[:], jvrep_i[:])

    # v2[p, (b, c, 0)] = global att row index b*T + c*128 + p ; col 1 <- g
    v2_i = const.tile([128, NB * 2], i32, tag="v2_i")
    nc.gpsimd.memset(v2_i[:], 0)
    nc.gpsimd.iota(
        v2_i[:].rearrange("p (b c two) -> p b c two", b=BL, c=TC)[:, :, :, 0],
        pattern=[[T, BL], [128, TC]], base=0, channel_multiplier=1,
    )
    v2 = const.tile([128, NB * 2], fp16, tag="v2")
    nc.gpsimd.tensor_copy(v2[:], v2_i[:])

    # tenrow[b, i] = 10*i  (for m = #{i<51 : 10i < seqlen})
    ten_i = const.tile([BL, KK], i32, tag="ten_i")
    nc.gpsimd.iota(ten_i[:], pattern=[[10, KK]], base=0, channel_multiplier=0)
    tenrow = const.tile([BL, KK], f32, tag="tenrow")
    nc.gpsimd.tensor_copy(tenrow[:], ten_i[:])

    # cand1[p] = LO0 + p*STEP1 ; iota2lo[p] = LO0 - STEP1 + p*STEP2
    pidx_i = const.tile([128, 1], i32, tag="pidx_i")
    nc.gpsimd.iota(pidx_i[:], pattern=[[0, 1]], base=0, channel_multiplier=1)
    pidx = const.tile([128, 1], f32, tag="pidx")
    nc.gpsimd.tensor_copy(pidx[:], pidx_i[:])
    tri = const.tile([128, 128], fp16, tag="tri")
    nc.gpsimd.tensor_scalar(
        out=tri[:], in0=qv[:], scalar1=pidx[:, 0:1], scalar2=None, op0=OP.is_ge
    )
    cand1 = const.tile([128, 1], f32, tag="cand1")
    nc.gpsimd.tensor_scalar(
        out=cand1[:], in0=pidx[:], scalar1=STEP1, scalar2=LO0, op0=OP.mult, op1=OP.add
    )
    iota2lo = const.tile([128, 1], f32, tag="iota2lo")
    nc.gpsimd.tensor_scalar(
        out=iota2lo[:], in0=pidx[:], scalar1=STEP2, scalar2=LO0 - STEP1,
        op0=OP.mult, op1=OP.add,
    )

    # ---------------- mask / seqlen / m  (needs only tgt) -------------------
    m16 = small.tile([128, NB], fp16, tag="m16")
    nc.vector.tensor_scalar(
        out=m16[:], in0=tgt_c[:], scalar1=0, scalar2=None, op0=OP.is_gt
    )
    nc.gpsimd.memset(
        m16[0:1, :].rearrange("p (b c) -> p b c", c=TC)[:, :, 0:1], 1.0
    )
    seqrow_ps = ps_sm.tile([1, NB], f32, tag="sm")
    nc.tensor.matmul(out=seqrow_ps[:], lhsT=ones_col[:], rhs=m16[:], start=True, stop=True)
    seqb = small.tile([1, BL], f32, tag="seqb")
    nc.vector.tensor_reduce(
        seqb[:].unsqueeze(2),
        seqrow_ps[:].rearrange("p (b c) -> p b c", c=TC),
        axis=AX.X, op=OP.add,
    )
    seqcol_ps = ps_sm.tile([BL, 1], f32, tag="sm")
    nc.tensor.transpose(seqcol_ps[:], seqb[:], id4[0:1, 0:1])
    mcol = small.tile([BL, 1], f32, tag="mcol")
    mcnt = small.tile([BL, KK], f32, tag="mcnt")
    nc.vector.tensor_scalar(
        out=mcnt[:], in0=tenrow[:], scalar1=seqcol_ps[:, 0:1], scalar2=None,
        op0=OP.is_lt, op1=OP.add, accum_out=mcol[:],
    )
    mrow_ps = ps_sm.tile([1, BL], f32, tag="sm")
    nc.tensor.transpose(mrow_ps[:], mcol[:], id4[:])
    mrow = small.tile([1, BL], f32, tag="mrow")
    nc.scalar.copy(mrow[:], mrow_ps[:])
    mrow16 = small.tile([1, BL], fp16, tag="mrow16")
    nc.gpsimd.tensor_copy(mrow16[:], mrow[:])
    mbc_ps = ps_sm.tile([128, BL], f32, tag="sm")
    nc.tensor.matmul(out=mbc_ps[:], lhsT=ones_row[:], rhs=mrow16[:], start=True, stop=True)
    mbc = small.tile([128, BL], f32, tag="mbc")
    nc.scalar.copy(mbc[:], mbc_ps[:])

    # ---------------- num / xn2 via ap-1 matmuls ----------------------------
    nx_ps = ps_nx.tile([128, 3 * NB * DC], f32, tag="nx")
    num4_ps = nx_ps[:, 0 : NB * DC]
    xn24_ps = nx_ps[:, NB * DC : 2 * NB * DC]
    pos4_ps = nx_ps[:, 2 * NB * DC : 3 * NB * DC]
    sqt = []
    for b in range(BL):
        sq = sqp.tile([128, DC * T], fp16, tag="sq")
        nc.vector.tensor_tensor(sq[:], embt[b][:], embt[b][:], op=OP.mult)
        sqt.append(sq)
    for b in range(BL):
        eR = embt[b][:].rearrange("p (dc t) -> p dc t", dc=DC)
        sR = sqt[b][:].rearrange("p (dc t) -> p dc t", dc=DC)
        for tcc in range(TC):
            col = (b * TC + tcc) * DC
            for dc in range(DC):
                nc.tensor.matmul(
                    out=num4_ps[:, col + dc : col + dc + 1],
                    lhsT=eR[:, dc, tcc * 128 : (tcc + 1) * 128],
                    rhs=fore_c[:, b * DC + dc : b * DC + dc + 1],
                    start=True, stop=True,
                )
            for dc in range(DC):
                nc.tensor.matmul(
                    out=xn24_ps[:, col + dc : col + dc + 1],
                    lhsT=sR[:, dc, tcc * 128 : (tcc + 1) * 128],
                    rhs=ones_col[:],
                    start=True, stop=True,
                )
    num_sb = small.tile([128, NB], f32, tag="num_sb")
    nc.vector.tensor_reduce(
        num_sb[:].unsqueeze(2),
        num4_ps.rearrange("p (col dc) -> p col dc", dc=DC),
        axis=AX.X, op=OP.add,
    )
    xn2_sb = small.tile([128, NB], f32, tag="xn2_sb")
    nc.vector.tensor_reduce(
        xn2_sb[:].unsqueeze(2),
        xn24_ps.rearrange("p (col dc) -> p col dc", dc=DC),
        axis=AX.X, op=OP.add,
    )

    # ---------------- w columns -------------------------------------------
    xn = small.tile([128, NB], f32, tag="xn")
    nc.scalar.sqrt(xn[:], xn2_sb[:])
    rs = small.tile([128, NB], f32, tag="rs")
    nc.vector.reciprocal(rs[:], xn[:])
    w32 = small.tile([128, NB], f32, tag="w32")
    nc.vector.tensor_tensor(w32[:], num_sb[:], rs[:], op=OP.mult)
    w16 = small.tile([128, NB], fp16, tag="w16")
    nc.vector.tensor_tensor(w16[:], w32[:], m16[:], op=OP.mult)

    # ---------------- w rows + broadcast tiles -----------------------------
    wbc16 = []
    for b in range(BL):
        wrow_ps = ps_sm.tile([BL, 128], fp16, tag="sm")
        nc.tensor.transpose(wrow_ps[:], w16[:, b * TC : (b + 1) * TC], id16[:])
        wrow_b = small.tile([BL, 128], fp16, tag=f"wrow{b}")
        nc.scalar.copy(wrow_b[:], wrow_ps[:])
        wps = ps_wbc.tile([128, T], f32, tag="wbc")
        for tcc in range(TC):
            nc.tensor.matmul(
                out=wps[:, tcc * 128 : (tcc + 1) * 128],
                lhsT=bcsel4[:, tcc * 128 : (tcc + 1) * 128],
                rhs=wrow_b[:],
                start=True, stop=True,
            )
        wsb = wbcp.tile([128, T], fp16, tag="wbc16")
        nc.scalar.copy(wsb[:], wps[:])
        wbc16.append(wsb)

    # ---------------- 2-round threshold search -----------------------------
    cnt1 = small.tile([128, BL], f32, tag="cnt1")
    scr1 = wbcp.tile([128, T], fp16, tag="scr")
    for b in range(BL):
        nc.vector.tensor_scalar(
            out=scr1[:], in0=wbc16[b][:], scalar1=cand1[:, 0:1], scalar2=None,
            op0=OP.is_ge, op1=OP.add, accum_out=cnt1[:, b : b + 1],
        )
    selc1 = small.tile([128, BL], fp16, tag="selc1")
    nc.vector.tensor_tensor(selc1[:], cnt1[:], mbc[:], op=OP.is_ge)
    n1_ps = ps_sm.tile([1, BL], f32, tag="sm")
    nc.tensor.matmul(out=n1_ps[:], lhsT=ones_col[:], rhs=selc1[:], start=True, stop=True)
    n1row = small.tile([1, BL], fp16, tag="n1row")
    nc.scalar.copy(n1row[:], n1_ps[:])
    n1bc_ps = ps_sm.tile([128, BL], f32, tag="sm")
    nc.tensor.matmul(out=n1bc_ps[:], lhsT=ones_row[:], rhs=n1row[:], start=True, stop=True)
    cand2 = small.tile([128, BL], f32, tag="cand2")
    nc.vector.tensor_scalar(
        out=cand2[:], in0=n1bc_ps[:], scalar1=STEP1, scalar2=iota2lo[:, 0:1],
        op0=OP.mult, op1=OP.add,
    )
    cnt2 = small.tile([128, BL], f32, tag="cnt2")
    for b in range(BL):
        nc.vector.tensor_scalar(
            out=scr1[:], in0=wbc16[b][:], scalar1=cand2[:, b : b + 1], scalar2=None,
            op0=OP.is_ge, op1=OP.add, accum_out=cnt2[:, b : b + 1],
        )
    selc2 = small.tile([128, BL], fp16, tag="selc2")
    nc.vector.tensor_tensor(selc2[:], cnt2[:], mbc[:], op=OP.is_ge)
    n2_ps = ps_sm.tile([1, BL], f32, tag="sm")
    nc.tensor.matmul(out=n2_ps[:], lhsT=ones_col[:], rhs=selc2[:], start=True, stop=True)
    # theta = LO0 + (n1-1)*STEP1 + (n2-1)*STEP2
    t1 = small.tile([1, BL], f32, tag="t1")
    nc.vector.tensor_scalar(
        out=t1[:], in0=n2_ps[:], scalar1=STEP2, scalar2=LO0 - STEP1 - STEP2,
        op0=OP.mult, op1=OP.add,
    )
    theta = small.tile([1, BL], f32, tag="theta")
    nc.vector.scalar_tensor_tensor(
        out=theta[:], in0=n1row[:], scalar=STEP1, in1=t1[:], op0=OP.mult, op1=OP.add
    )
    thbc_ps = ps_sm.tile([128, BL], f32, tag="sm")
    nc.tensor.matmul(out=thbc_ps[:], lhsT=ones_row32[:], rhs=theta[:], start=True, stop=True)

    # ---------------- selection, prefix, one-hot compaction -----------------
    sel16 = small.tile([128, NB], fp16, tag="sel16")
    nc.vector.tensor_tensor(
        sel16[:].rearrange("p (b c) -> p b c", c=TC),
        w16[:].rearrange("p (b c) -> p b c", c=TC),
        thbc_ps[:].unsqueeze(2).broadcast_to([128, BL, TC]),
        op=OP.is_ge,
    )
    g16 = small.tile([128, NB], fp16, tag="g16")
    nc.vector.scalar_tensor_tensor(
        out=g16[:], in0=w16[:], scalar=0.0, in1=sel16[:], op0=OP.max, op1=OP.mult
    )
    nc.gpsimd.tensor_copy(
        v2[:].rearrange("p (b c two) -> p b c two", b=BL, c=TC)[:, :, :, 1],
        g16[:].rearrange("p (b c) -> p b c", c=TC),
    )

    # pos[q, (b,c)] = prefix count of sel up to global position (c*128+q)
    # pos4 col ((b,c), k): k=0 -> TRI@sel[c]; k>=1 -> ONES@sel[c-k] (zero pad)
    selR = sel16[:].rearrange("p (b c) -> p b c", c=TC)
    pos4R = pos4_ps.rearrange("p (col k) -> p col k", k=DC)
    for b in range(BL):
        for c in range(TC):
            nc.tensor.matmul(
                out=pos4R[:, b * TC + c, 0:1],
                lhsT=tri[:], rhs=selR[:, b, c].unsqueeze(1),
                start=True, stop=True,
            )
    for b in range(BL):
        for c in range(TC):
            for k in range(1, DC):
                rhs = (
                    selR[:, b, c - k].unsqueeze(1) if c - k >= 0 else zero_col[:]
                )
                nc.tensor.matmul(
                    out=pos4R[:, b * TC + c, k : k + 1],
                    lhsT=ones128[:], rhs=rhs,
                    start=True, stop=True,
                )
    pos_sb = small.tile([128, NB], f32, tag="pos_sb")
    nc.vector.tensor_reduce(
        pos_sb[:].unsqueeze(2),
        pos4R,
        axis=AX.X, op=OP.add,
    )
    posm = small.tile([128, NB], fp16, tag="posm")
    nc.vector.tensor_tensor(posm[:], pos_sb[:], sel16[:], op=OP.mult)

    # st[p, (b, j, c)] = [posm[p, (b,c)] == j+1]
    st4 = small.tile([128, BL * J * TC], fp16, tag="st4")
    nc.vector.tensor_tensor(
        out=st4[:].rearrange("p (b j c) -> p b j c", b=BL, j=J),
        in0=posm[:].rearrange("p (b c) -> p b c", c=TC).unsqueeze(2).broadcast_to([128, BL, J, TC]),
        in1=jvrep[:].rearrange("p (j c) -> p j c", j=J).unsqueeze(1).broadcast_to([128, BL, J, TC]),
        op=OP.is_equal,
    )
    st4R = st4[:].rearrange("p (b j c) -> p b j c", b=BL, j=J)
    v2R = v2[:].rearrange("p (b c two) -> p b c two", b=BL, c=TC)
    staks = []
    for b in range(BL):
        stak_ps = ps_sm.tile([2, J], f32, tag="sm")
        for c in range(TC):
            nc.tensor.matmul(
                out=stak_ps[:],
                lhsT=v2R[:, b, c, :],
                rhs=st4R[:, b, :, c],
                start=(c == 0), stop=(c == TC - 1),
            )
        stak_b = small.tile([2, J], f32, tag=f"stak{b}")
        nc.scalar.copy(stak_b[:], stak_ps[:])
        staks.append(stak_b)

    # ---------------- per-sample gather + tot ------------------------------
    gsel = small.tile([J, BL], fp16, tag="gsel")
    nc.gpsimd.memset(gsel[:], 0.0)
    totAB_ps = ps_tot.tile([128, 2 * BL * H], f32, tag="totAB")
    totA_ps = totAB_ps[:, 0 : BL * H]
    totB_ps = totAB_ps[0 : S - 128, BL * H : 2 * BL * H]
    for b in range(BL):
        pst_ps = ps_sm.tile([J, 2], f32, tag="sm")
        nc.tensor.transpose(pst_ps[:], staks[b][:], id4[0:2, 0:2])
        idx_b = small.tile([J, 1], i32, tag=f"idx{b}")
        nc.scalar.copy(idx_b[:], pst_ps[:, 0:1])
        nc.scalar.copy(gselb[b][:, b : b + 1], pst_ps[:, 1:2])
        gat_b = gatp.tile([J, HS], fp16, tag="gat")
        nc.gpsimd.indirect_dma_start(
            out=gat_b[:],
            out_offset=None,
            in_=att[:],
            in_offset=bass.IndirectOffsetOnAxis(ap=idx_b[:, 0:1], axis=0),
        )
        for h in range(H):
            nc.tensor.matmul(
                out=totA_ps[:, b * H + h : b * H + h + 1],
                lhsT=gat_b[:, h * S : h * S + 128],
                rhs=gsel[:, b : b + 1],
                start=True, stop=True,
            )
        for h in range(H):
            nc.tensor.matmul(
                out=totB_ps[:, b * H + h : b * H + h + 1],
                lhsT=gat_b[:, h * S + 128 : (h + 1) * S],
                rhs=gsel[:, b : b + 1],
                start=True, stop=True,
            )

    # ---------------- transpose back + normalize ---------------------------
    totA32 = small.tile([128, BL], f32, tag="totA32")
    nc.vector.tensor_reduce(
        totA32[:].unsqueeze(2),
        totA_ps.rearrange("p (b h) -> p b h", h=H),
        axis=AX.X, op=OP.add,
    )
    totB32 = small.tile([S - 128, BL], f32, tag="totB32")
    nc.vector.tensor_reduce(
        totB32[:].unsqueeze(2),
        totB_ps.rearrange("p (b h) -> p b h", h=H),
        axis=AX.X, op=OP.add,
    )
    totA16 = small.tile([128, BL], fp16, tag="totA16")
    nc.scalar.copy(totA16[:], totA32[:])
    totB16 = small.tile([S - 128, BL], fp16, tag="totB16")
    nc.scalar.copy(totB16[:], totB32[:])
    trAB_ps = ps_tot.tile([BL, S], fp16, tag="trAB")
    nc.tensor.transpose(trAB_ps[:, 0:128], totA16[:], id16[:])
    nc.tensor.transpose(trAB_ps[:, 128:S], totB16[:], id16[0 : S - 128, 0 : S - 128])
    tot16 = small.tile([BL, S], fp16, tag="tot16")
    nc.scalar.copy(tot16[:], trAB_ps[:])

    mn = small.tile([BL, 1], f32, tag="mn")
    nc.vector.tensor_reduce(mn[:], tot16[:], axis=AX.X, op=OP.min)
    mx = small.tile([BL, 1], f32, tag="mx")
    nc.vector.tensor_reduce(mx[:], tot16[:], axis=AX.X, op=OP.max)
    rng = small.tile([BL, 1], f32, tag="rng")
    nc.gpsimd.tensor_tensor(rng[:], mx[:], mn[:], op=OP.subtract)
    rinv = small.tile([BL, 1], f32, tag="rinv")
    nc.vector.reciprocal(rinv[:], rng[:])
    out_sb = small.tile([BL, S], f32, tag="out_sb")
    nc.vector.tensor_scalar(
        out=out_sb[:], in0=tot16[:], scalar1=mn[:, 0:1], scalar2=rinv[:, 0:1],
        op0=OP.subtract, op1=OP.mult,
    )
    nc.sync.dma_start(out[:], out_sb[:])


def build_nc(path=None):
    nc = bacc.Bacc("TRN2", target_bir_lowering=False, debug=False)
    emb = nc.dram_tensor("emb", [BL, D, T], fp16, kind="ExternalInput")
    att = nc.dram_tensor("att", [BL * T, HS], fp16, kind="ExternalInput")
    fore = nc.dram_tensor("fore", [128, BL * DC], fp16, kind="ExternalInput")
    tgtT = nc.dram_tensor("tgtT", [128, NB], i32, kind="ExternalInput")
    out = nc.dram_tensor("out", [BL, S], f32, kind="ExternalOutput")
    with ExitStack() as ctx:
        tc_ = ctx.enter_context(tile.TileContext(nc))
        build_body(ctx, tc_, emb.ap(), att.ap(), fore.ap(), tgtT.ap(), out.ap())
    nc.compile()
    return nc


_NC_CACHE = {}


def get_nc(path=None):
    if "nc" not in _NC_CACHE:
        _NC_CACHE["nc"] = build_nc()
    return _NC_CACHE["nc"]


def make_in_maps(fore_rep_encoded, target_embed, align_attns, targets):
    LAYER_ID = 2
    att_l = np.transpose(np.asarray(align_attns[LAYER_ID]), (0, 2, 1, 3))  # [B,T,H,S]
    att_l = np.ascontiguousarray(att_l, dtype=np.float16)
    emb_d = np.ascontiguousarray(
        np.swapaxes(np.asarray(target_embed), 1, 2), dtype=np.float16
    )  # [B, D, T]
    fore_np = np.asarray(fore_rep_encoded, dtype=np.float16)  # [B, D]
    tgt_np = np.asarray(targets)[:, :T].astype(np.int32)      # [B, T]
    in_maps = []
    for cidx in range(NCORES):
        sl = slice(cidx * BL, (cidx + 1) * BL)
        fore_sl = fore_np[sl]                      # [BL, D]
        # fore cols [(p), (b, dc)]: col b*DC+dc = fore[b, dc*128:(dc+1)*128]
        fore_c = np.ascontiguousarray(
            fore_sl.reshape(BL, DC, 128).transpose(2, 0, 1).reshape(128, BL * DC)
        )
        tgt_sl = tgt_np[sl]                        # [BL, T]
        # tgtT [(p), (b, c)]: col b*TC+c = tgt[b, c*128:(c+1)*128]
        tgt_T = np.ascontiguousarray(
            tgt_sl.reshape(BL, TC, 128).transpose(2, 0, 1).reshape(128, NB)
        )
        in_maps.append(
            {
                "emb": np.ascontiguousarray(emb_d[sl]),
                "att": att_l[sl].reshape(BL * T, HS),
                "fore": fore_c,
                "tgtT": tgt_T,
            }
        )
    return in_maps


def kernel(fore_rep_encoded, target_embed, align_attns, targets):
    global LAST_EXEC_NS, LAST_RESULTS
    nc = get_nc()
    in_maps = make_in_maps(fore_rep_encoded, target_embed, align_attns, targets)
    trace = bool(os.environ.get("KERNEL_TRACE"))
    try:
        res = bass_utils.run_bass_kernel_spmd(
            nc, in_maps, core_ids=list(range(NCORES)), trace=trace
        )
    except ModuleNotFoundError:
        os.environ["BASS_NEVER_TRACE"] = "1"
        res = bass_utils.run_bass_kernel_spmd(
            nc, in_maps, core_ids=list(range(NCORES)), trace=False
        )
    LAST_EXEC_NS = res.exec_time_ns
    LAST_RESULTS = res
    return np.concatenate([r["out"] for r in res.results], axis=0)


# revision 9
# speedup vs baseline: 1.0633x; 1.0633x over previous
"""Trainium2 Bass kernel v2 for nn_CamAttnCon (topk-masked CAM attention).

Strategy (per core, 4 samples, D-layout fp16):
  - emb shipped host-transposed [BL, D, T] fp16; one big DMA per sample.
  - num[t] = e.f via ap-1 PE matmuls (lhsT = emb chunk [128D,128T], rhs = fore
    col) -> out [128,1] PSUM, nearly free in the cost model.
  - xn2[t] via tensor_scalar pow(e,2) (one DVE op per sample) + ap-1 ones
    matmuls.
  - w = num * rsqrt(xn2), masked; fp16 w16.  Selection threshold theta found
    by a 2-round 128-candidate parallel count search on a broadcast w (exact
    because round-2 step < fp16 ulp at theta).
  - Compaction by prefix-sum of the selection mask (triangular + ones PE
    matmuls), tie-robust; one-hot is_equal -> compact (row index, gather
    weight) pairs via PE; indirect-DMA gather of selected fp16 attn rows.
  - tot[s] = sum_j g_j att[t_j, h, s] over h via ap-1 matmuls in s-partition
    space; transpose back; min-max normalize.
"""

import os
import sys

sys.path.insert(0, "/opt/trn_rl_repo")

import numpy as np
from contextlib import ExitStack

import concourse.bass as bass
import concourse.bacc as bacc
import concourse.mybir as mybir
import concourse.tile as tile
from concourse.masks import make_identity
from concourse import bass_utils

f32 = mybir.dt.float32
fp16 = mybir.dt.float16
i32 = mybir.dt.int32
AX = mybir.AxisListType
OP = mybir.AluOpType
AF = mybir.ActivationFunctionType

B, T, D, H, S = 32, 512, 512, 8, 196
NCORES = 8
BL = B // NCORES            # 4 samples per core
TC = T // 128               # 4 t-chunks of 128
DC = D // 128               # 4 d-chunks of 128
HS = H * S                  # 1568
KK = 51                     # max top-k count
J = 52                      # padded slot count (>= KK, slots 1..52 hold pos)
NB = BL * TC                # 16 (b,tc) columns

# threshold search grid: theta in [LO0, LO0 + 128*STEP1)
LO0 = 0.0
STEP1 = 4.0 / 128.0         # 0.03125
STEP2 = STEP1 / 128.0       # 2.44e-4 < fp16 ulp at theta (~9.8e-4)

LAST_EXEC_NS = None
LAST_RESULTS = None


def build_body(ctx, tc_, emb, att, fore, tgtT, out):
    nc = tc_.nc

    const = ctx.enter_context(tc_.tile_pool(name="const", bufs=1))
    small = ctx.enter_context(tc_.tile_pool(name="small", bufs=1))
    embp = ctx.enter_context(tc_.tile_pool(name="embp", bufs=4))
    sqp = ctx.enter_context(tc_.tile_pool(name="sqp", bufs=2))
    wbcp = ctx.enter_context(tc_.tile_pool(name="wbcp", bufs=4))
    gatp = ctx.enter_context(tc_.tile_pool(name="gatp", bufs=4))

    ps_nx = ctx.enter_context(tc_.tile_pool(name="ps_nx", bufs=1, space="PSUM"))
    ps_wbc = ctx.enter_context(tc_.tile_pool(name="ps_wbc", bufs=1, space="PSUM"))
    ps_sm = ctx.enter_context(tc_.tile_pool(name="ps_sm", bufs=2, space="PSUM"))
    ps_tot = ctx.enter_context(tc_.tile_pool(name="ps_tot", bufs=1, space="PSUM"))

    # ---------------- input DMAs (tgt/fore first: tiny; then emb) -----------
    tgt_c = small.tile([128, NB], i32, tag="tgt_c")
    nc.sync.dma_start(tgt_c[:], tgtT[:])
    fore_c = small.tile([128, BL * DC], fp16, tag="fore_c")
    nc.scalar.dma_start(fore_c[:], fore[:])
    fore_c = small.tile([128, BL * DC], fp16, tag="fore_c")
    nc.sync.dma_start(fore_c[:], fore[:])
    embR = emb.rearrange("b (dc p) t -> b p dc t", p=128)
    embt = []
    for b in range(BL):
        e = embp.tile([128, DC * T], fp16, tag="emb")
        nc.sync.dma_start(e[:].rearrange("p (dc t) -> p dc t", dc=DC), embR[b])
        embt.append(e)

    # ---------------- constants (Pool/ACT, overlap the DMAs) ----------------
    # warm ACT function tables used later (rsqrt / reciprocal / copy)
    warm = const.tile([1, 1], f32, tag="warm")
    nc.vector.memset(warm[:], 1.0)
    warm2 = const.tile([1, 1], f32, tag="warm2")
    nc.scalar.sqrt(warm2[:], warm[:])
    nc.scalar.copy(warm2[:], warm[:])

    ones_col = const.tile([128, 1], fp16, tag="ones_col")
    nc.gpsimd.memset(ones_col[:], 1.0)
    ones_row = const.tile([1, 128], fp16, tag="ones_row")
    nc.gpsimd.memset(ones_row[:], 1.0)
    ones_row32 = const.tile([1, 128], f32, tag="ones_row32")
    nc.gpsimd.memset(ones_row32[:], 1.0)
    ones128 = const.tile([128, 128], fp16, tag="ones128")
    nc.gpsimd.memset(ones128[:], 1.0)
    zero_col = const.tile([128, 1], fp16, tag="zero_col")
    nc.gpsimd.memset(zero_col[:], 0.0)
    # bcsel4: slice tc [4, 128] has row tc all-ones (partition-bcast selector)
    bcsel4 = const.tile([BL, TC * 128], fp16, tag="bcsel4")
    nc.gpsimd.memset(bcsel4[:], 0.0)
    nc.gpsimd.affine_select(
        out=bcsel4[:].rearrange("p (blk j) -> p blk j", blk=TC),
        in_=bcsel4[:].rearrange("p (blk j) -> p blk j", blk=TC),
        compare_op=OP.not_equal,
        fill=1.0,
        base=0,
        pattern=[[-1, TC], [0, 128]],
        channel_multiplier=1,
    )

    # TRI[p, q] = 1 if p <= q  (inclusive prefix along partitions)
    qv_i = const.tile([128, 128], i32, tag="qv_i")
    nc.gpsimd.iota(qv_i[:], pattern=[[1, 128]], base=0, channel_multiplier=0)
    qv = const.tile([128, 128], f32, tag="qv")
    nc.gpsimd.tensor_copy(qv[:], qv_i[:])

    id16 = const.tile([128, 128], fp16, tag="id16")
    nc.gpsimd.memset(id16[:], 0.0)
    nc.gpsimd.affine_select(
        out=id16[:], in_=id16[:], compare_op=OP.not_equal, fill=1.0,
        base=0, pattern=[[1, 128]], channel_multiplier=-1,
    )
    id4 = const.tile([4, 4], f32, tag="id4")
    make_identity(nc, id4[:])

    # jvrep[p, (j, c)] = j + 1   (slot match values, c-packed for 2x mode)
    jvrep_i = const.tile([128, J * TC], i32, tag="jvrep_i")
    nc.gpsimd.iota(jvrep_i[:], pattern=[[1, J], [0, TC]], base=1, channel_multiplier=0)
    jvrep = const.tile([128, J * TC], fp16, tag="jvrep")
    nc.gpsimd.tensor_copy(jvrep[:], jvrep_i[:])

    # v2[p, (b, c, 0)] = global att row index b*T + c*128 + p ; col 1 <- g
    v2_i = const.tile([128, NB * 2], i32, tag="v2_i")
    nc.gpsimd.memset(v2_i[:], 0)
    nc.gpsimd.iota(
        v2_i[:].rearrange("p (b c two) -> p b c two", b=BL, c=TC)[:, :, :, 0],
        pattern=[[T, BL], [128, TC]], base=0, channel_multiplier=1,
    )
    v2 = const.tile([128, NB * 2], fp16, tag="v2")
    nc.gpsimd.tensor_copy(v2[:], v2_i[:])

    # tenrow[b, i] = 10*i  (for m = #{i<51 : 10i < seqlen})
    ten_i = const.tile([BL, KK], i32, tag="ten_i")
    nc.gpsimd.iota(ten_i[:], pattern=[[10, KK]], base=0, channel_multiplier=0)
    tenrow = const.tile([BL, KK], f32, tag="tenrow")
    nc.gpsimd.tensor_copy(tenrow[:], ten_i[:])

    # cand1[p] = LO0 + p*STEP1 ; iota2lo[p] = LO0 - STEP1 + p*STEP2
    pidx_i = const.tile([128, 1], i32, tag="pidx_i")
    nc.gpsimd.iota(pidx_i[:], pattern=[[0, 1]], base=0, channel_multiplier=1)
    pidx = const.tile([128, 1], f32, tag="pidx")
    nc.gpsimd.tensor_copy(pidx[:], pidx_i[:])
    tri = const.tile([128, 128], fp16, tag="tri")
    nc.gpsimd.tensor_scalar(
        out=tri[:], in0=qv[:], scalar1=pidx[:, 0:1], scalar2=None, op0=OP.is_ge
    )
    cand1 = const.tile([128, 1], f32, tag="cand1")
    nc.gpsimd.tensor_scalar(
        out=cand1[:], in0=pidx[:], scalar1=STEP1, scalar2=LO0, op0=OP.mult, op1=OP.add
    )
    iota2lo = const.tile([128, 1], f32, tag="iota2lo")
    nc.gpsimd.tensor_scalar(
        out=iota2lo[:], in0=pidx[:], scalar1=STEP2, scalar2=LO0 - STEP1,
        op0=OP.mult, op1=OP.add,
    )

    tgt_c = small.tile([128, NB], i32, tag="tgt_c")
    nc.gpsimd.dma_start(tgt_c[:], tgtT[:])
    # ---------------- mask / seqlen / m  (needs only tgt) -------------------
    m16 = small.tile([128, NB], fp16, tag="m16")
    nc.vector.tensor_scalar(
        out=m16[:], in0=tgt_c[:], scalar1=0, scalar2=None, op0=OP.is_gt
    )
    nc.gpsimd.memset(
        m16[0:1, :].rearrange("p (b c) -> p b c", c=TC)[:, :, 0:1], 1.0
    )
    seqrow_ps = ps_sm.tile([1, NB], f32, tag="sm")
    nc.tensor.matmul(out=seqrow_ps[:], lhsT=ones_col[:], rhs=m16[:], start=True, stop=True)
    seqb = small.tile([1, BL], f32, tag="seqb")
    nc.vector.tensor_reduce(
        seqb[:].unsqueeze(2),
        seqrow_ps[:].rearrange("p (b c) -> p b c", c=TC),
        axis=AX.X, op=OP.add,
    )
    seqcol_ps = ps_sm.tile([BL, 1], f32, tag="sm")
    nc.tensor.transpose(seqcol_ps[:], seqb[:], id4[0:1, 0:1])
    mcol = small.tile([BL, 1], f32, tag="mcol")
    mcnt = small.tile([BL, KK], f32, tag="mcnt")
    nc.vector.tensor_scalar(
        out=mcnt[:], in0=tenrow[:], scalar1=seqcol_ps[:, 0:1], scalar2=None,
        op0=OP.is_lt, op1=OP.add, accum_out=mcol[:],
    )
    mrow_ps = ps_sm.tile([1, BL], f32, tag="sm")
    nc.tensor.transpose(mrow_ps[:], mcol[:], id4[:])
    mrow = small.tile([1, BL], f32, tag="mrow")
    nc.scalar.copy(mrow[:], mrow_ps[:])
    mrow16 = small.tile([1, BL], fp16, tag="mrow16")
    nc.gpsimd.tensor_copy(mrow16[:], mrow[:])
    mbc_ps = ps_sm.tile([128, BL], f32, tag="sm")
    nc.tensor.matmul(out=mbc_ps[:], lhsT=ones_row[:], rhs=mrow16[:], start=True, stop=True)
    mbc = small.tile([128, BL], f32, tag="mbc")
    nc.scalar.copy(mbc[:], mbc_ps[:])

    # ---------------- num / xn2 via ap-1 matmuls ----------------------------
    nx_ps = ps_nx.tile([128, 3 * NB * DC], f32, tag="nx")
    num4_ps = nx_ps[:, 0 : NB * DC]
    xn24_ps = nx_ps[:, NB * DC : 2 * NB * DC]
    pos4_ps = nx_ps[:, 2 * NB * DC : 3 * NB * DC]
    sqt = []
    for b in range(BL):
        sq = sqp.tile([128, DC * T], fp16, tag="sq")
        nc.vector.tensor_tensor(sq[:], embt[b][:], embt[b][:], op=OP.mult)
        sqt.append(sq)
    for b in range(BL):
        eR = embt[b][:].rearrange("p (dc t) -> p dc t", dc=DC)
        sR = sqt[b][:].rearrange("p (dc t) -> p dc t", dc=DC)
        for tcc in range(TC):
            col = (b * TC + tcc) * DC
            for dc in range(DC):
                nc.tensor.matmul(
                    out=num4_ps[:, col + dc : col + dc + 1],
                    lhsT=eR[:, dc, tcc * 128 : (tcc + 1) * 128],
                    rhs=fore_c[:, b * DC + dc : b * DC + dc + 1],
                    start=True, stop=True,
                )
            for dc in range(DC):
                nc.tensor.matmul(
                    out=xn24_ps[:, col + dc : col + dc + 1],
                    lhsT=sR[:, dc, tcc * 128 : (tcc + 1) * 128],
                    rhs=ones_col[:],
                    start=True, stop=True,
                )
    num_sb = small.tile([128, NB], f32, tag="num_sb")
    nc.vector.tensor_reduce(
        num_sb[:].unsqueeze(2),
        num4_ps.rearrange("p (col dc) -> p col dc", dc=DC),
        axis=AX.X, op=OP.add,
    )
    xn2_sb = small.tile([128, NB], f32, tag="xn2_sb")
    nc.vector.tensor_reduce(
        xn2_sb[:].unsqueeze(2),
        xn24_ps.rearrange("p (col dc) -> p col dc", dc=DC),
        axis=AX.X, op=OP.add,
    )

    # ---------------- w columns -------------------------------------------
    xn = small.tile([128, NB], f32, tag="xn")
    nc.scalar.sqrt(xn[:], xn2_sb[:])
    rs = small.tile([128, NB], f32, tag="rs")
    nc.vector.reciprocal(rs[:], xn[:])
    w32 = small.tile([128, NB], f32, tag="w32")
    nc.vector.tensor_tensor(w32[:], num_sb[:], rs[:], op=OP.mult)
    w16 = small.tile([128, NB], fp16, tag="w16")
    nc.vector.tensor_tensor(w16[:], w32[:], m16[:], op=OP.mult)

    # ---------------- w rows + broadcast tiles -----------------------------
    wbc16 = []
    for b in range(BL):
        wrow_ps = ps_sm.tile([BL, 128], fp16, tag="sm")
        nc.tensor.transpose(wrow_ps[:], w16[:, b * TC : (b + 1) * TC], id16[:])
        wrow_b = small.tile([BL, 128], fp16, tag=f"wrow{b}")
        nc.scalar.copy(wrow_b[:], wrow_ps[:])
        wps = ps_wbc.tile([128, T], f32, tag="wbc")
        for tcc in range(TC):
            nc.tensor.matmul(
                out=wps[:, tcc * 128 : (tcc + 1) * 128],
                lhsT=bcsel4[:, tcc * 128 : (tcc + 1) * 128],
                rhs=wrow_b[:],
                start=True, stop=True,
            )
        wsb = wbcp.tile([128, T], fp16, tag="wbc16")
        nc.scalar.copy(wsb[:], wps[:])
        wbc16.append(wsb)

    # ---------------- 2-round threshold search -----------------------------
    cnt1 = small.tile([128, BL], f32, tag="cnt1")
    scr1 = wbcp.tile([128, T], fp16, tag="scr")
    for b in range(BL):
        nc.vector.tensor_scalar(
            out=scr1[:], in0=wbc16[b][:], scalar1=cand1[:, 0:1], scalar2=None,
            op0=OP.is_ge, op1=OP.add, accum_out=cnt1[:, b : b + 1],
        )
    selc1 = small.tile([128, BL], fp16, tag="selc1")
    nc.vector.tensor_tensor(selc1[:], cnt1[:], mbc[:], op=OP.is_ge)
    n1_ps = ps_sm.tile([1, BL], f32, tag="sm")
    nc.tensor.matmul(out=n1_ps[:], lhsT=ones_col[:], rhs=selc1[:], start=True, stop=True)
    n1row = small.tile([1, BL], fp16, tag="n1row")
    nc.scalar.copy(n1row[:], n1_ps[:])
    n1bc_ps = ps_sm.tile([128, BL], f32, tag="sm")
    nc.tensor.matmul(out=n1bc_ps[:], lhsT=ones_row[:], rhs=n1row[:], start=True, stop=True)
    cand2 = small.tile([128, BL], f32, tag="cand2")
    nc.vector.tensor_scalar(
        out=cand2[:], in0=n1bc_ps[:], scalar1=STEP1, scalar2=iota2lo[:, 0:1],
        op0=OP.mult, op1=OP.add,
    )
    cnt2 = small.tile([128, BL], f32, tag="cnt2")
    for b in range(BL):
        nc.vector.tensor_scalar(
            out=scr1[:], in0=wbc16[b][:], scalar1=cand2[:, b : b + 1], scalar2=None,
            op0=OP.is_ge, op1=OP.add, accum_out=cnt2[:, b : b + 1],
        )
    selc2 = small.tile([128, BL], fp16, tag="selc2")
    nc.vector.tensor_tensor(selc2[:], cnt2[:], mbc[:], op=OP.is_ge)
    n2_ps = ps_sm.tile([1, BL], f32, tag="sm")
    nc.tensor.matmul(out=n2_ps[:], lhsT=ones_col[:], rhs=selc2[:], start=True, stop=True)
    # theta = LO0 + (n1-1)*STEP1 + (n2-1)*STEP2
    t1 = small.tile([1, BL], f32, tag="t1")
    nc.vector.tensor_scalar(
        out=t1[:], in0=n2_ps[:], scalar1=STEP2, scalar2=LO0 - STEP1 - STEP2,
        op0=OP.mult, op1=OP.add,
    )
    theta = small.tile([1, BL], f32, tag="theta")
    nc.vector.scalar_tensor_tensor(
        out=theta[:], in0=n1row[:], scalar=STEP1, in1=t1[:], op0=OP.mult, op1=OP.add
    )
    thbc_ps = ps_sm.tile([128, BL], f32, tag="sm")
    nc.tensor.matmul(out=thbc_ps[:], lhsT=ones_row32[:], rhs=theta[:], start=True, stop=True)

    # ---------------- selection, prefix, one-hot compaction -----------------
    sel16 = small.tile([128, NB], fp16, tag="sel16")
    nc.vector.tensor_tensor(
        sel16[:].rearrange("p (b c) -> p b c", c=TC),
        w16[:].rearrange("p (b c) -> p b c", c=TC),
        thbc_ps[:].unsqueeze(2).broadcast_to([128, BL, TC]),
        op=OP.is_ge,
    )
    g16 = small.tile([128, NB], fp16, tag="g16")
    nc.vector.scalar_tensor_tensor(
        out=g16[:], in0=w16[:], scalar=0.0, in1=sel16[:], op0=OP.max, op1=OP.mult
    )
    nc.gpsimd.tensor_copy(
        v2[:].rearrange("p (b c two) -> p b c two", b=BL, c=TC)[:, :, :, 1],
        g16[:].rearrange("p (b c) -> p b c", c=TC),
    )

    # pos[q, (b,c)] = prefix count of sel up to global position (c*128+q)
    # pos4 col ((b,c), k): k=0 -> TRI@sel[c]; k>=1 -> ONES@sel[c-k] (zero pad)
    selR = sel16[:].rearrange("p (b c) -> p b c", c=TC)
    pos4R = pos4_ps.rearrange("p (col k) -> p col k", k=DC)
    for b in range(BL):
        for c in range(TC):
            nc.tensor.matmul(
                out=pos4R[:, b * TC + c, 0:1],
                lhsT=tri[:], rhs=selR[:, b, c].unsqueeze(1),
                start=True, stop=True,
            )
    for b in range(BL):
        for c in range(TC):
            for k in range(1, DC):
                rhs = (
                    selR[:, b, c - k].unsqueeze(1) if c - k >= 0 else zero_col[:]
                )
                nc.tensor.matmul(
                    out=pos4R[:, b * TC + c, k : k + 1],
                    lhsT=ones128[:], rhs=rhs,
                    start=True, stop=True,
                )
    pos_sb = small.tile([128, NB], f32, tag="pos_sb")
    nc.vector.tensor_reduce(
        pos_sb[:].unsqueeze(2),
        pos4R,
        axis=AX.X, op=OP.add,
    )
    posm = small.tile([128, NB], fp16, tag="posm")
    nc.vector.tensor_tensor(posm[:], pos_sb[:], sel16[:], op=OP.mult)

    # st[p, (b, j, c)] = [posm[p, (b,c)] == j+1]
    st4 = small.tile([128, BL * J * TC], fp16, tag="st4")
    nc.vector.tensor_tensor(
        out=st4[:].rearrange("p (b j c) -> p b j c", b=BL, j=J),
        in0=posm[:].rearrange("p (b c) -> p b c", c=TC).unsqueeze(2).broadcast_to([128, BL, J, TC]),
        in1=jvrep[:].rearrange("p (j c) -> p j c", j=J).unsqueeze(1).broadcast_to([128, BL, J, TC]),
        op=OP.is_equal,
    )
    st4R = st4[:].rearrange("p (b j c) -> p b j c", b=BL, j=J)
    v2R = v2[:].rearrange("p (b c two) -> p b c two", b=BL, c=TC)
    staks = []
    for b in range(BL):
        stak_ps = ps_sm.tile([2, J], f32, tag="sm")
        for c in range(TC):
            nc.tensor.matmul(
                out=stak_ps[:],
                lhsT=v2R[:, b, c, :],
                rhs=st4R[:, b, :, c],
                start=(c == 0), stop=(c == TC - 1),
            )
        stak_b = small.tile([2, J], f32, tag=f"stak{b}")
        nc.scalar.copy(stak_b[:], stak_ps[:])
        staks.append(stak_b)

    # ---------------- per-sample gather + tot ------------------------------
    gsel = small.tile([J, BL], fp16, tag="gsel")
    nc.gpsimd.memset(gsel[:], 0.0)
    totAB_ps = ps_tot.tile([128, 2 * BL * H], f32, tag="totAB")
    totA_ps = totAB_ps[:, 0 : BL * H]
    totB_ps = totAB_ps[0 : S - 128, BL * H : 2 * BL * H]
    for b in range(BL):
        pst_ps = ps_sm.tile([J, 2], f32, tag="sm")
        nc.tensor.transpose(pst_ps[:], staks[b][:], id4[0:2, 0:2])
        idx_b = small.tile([J, 1], i32, tag=f"idx{b}")
        nc.scalar.copy(idx_b[:], pst_ps[:, 0:1])
        nc.scalar.copy(gselb[b][:, b : b + 1], pst_ps[:, 1:2])
        gat_b = gatp.tile([J, HS], fp16, tag="gat")
        nc.gpsimd.indirect_dma_start(
            out=gat_b[:],
            out_offset=None,
            in_=att[:],
            in_offset=bass.IndirectOffsetOnAxis(ap=idx_b[:, 0:1], axis=0),
        )
        for h in range(H):
            nc.tensor.matmul(
                out=totA_ps[:, b * H + h : b * H + h + 1],
                lhsT=gat_b[:, h * S : h * S + 128],
                rhs=gsel[:, b : b + 1],
                start=True, stop=True,
            )
        for h in range(H):
            nc.tensor.matmul(
                out=totB_ps[:, b * H + h : b * H + h + 1],
                lhsT=gat_b[:, h * S + 128 : (h + 1) * S],
                rhs=gsel[:, b : b + 1],
                start=True, stop=True,
            )

    # ---------------- transpose back + normalize ---------------------------
    totA32 = small.tile([128, BL], f32, tag="totA32")
    nc.vector.tensor_reduce(
        totA32[:].unsqueeze(2),
        totA_ps.rearrange("p (b h) -> p b h", h=H),
        axis=AX.X, op=OP.add,
    )
    totB32 = small.tile([S - 128, BL], f32, tag="totB32")
    nc.vector.tensor_reduce(
        totB32[:].unsqueeze(2),
        totB_ps.rearrange("p (b h) -> p b h", h=H),
        axis=AX.X, op=OP.add,
    )
    totA16 = small.tile([128, BL], fp16, tag="totA16")
    nc.scalar.copy(totA16[:], totA32[:])
    totB16 = small.tile([S - 128, BL], fp16, tag="totB16")
    nc.scalar.copy(totB16[:], totB32[:])
    trAB_ps = ps_tot.tile([BL, S], fp16, tag="trAB")
    nc.tensor.transpose(trAB_ps[:, 0:128], totA16[:], id16[:])
    nc.tensor.transpose(trAB_ps[:, 128:S], totB16[:], id16[0 : S - 128, 0 : S - 128])
    tot16 = small.tile([BL, S], fp16, tag="tot16")
    nc.scalar.copy(tot16[:], trAB_ps[:])

    mn = small.tile([BL, 1], f32, tag="mn")
    nc.vector.tensor_reduce(mn[:], tot16[:], axis=AX.X, op=OP.min)
    mx = small.tile([BL, 1], f32, tag="mx")
    nc.vector.tensor_reduce(mx[:], tot16[:], axis=AX.X, op=OP.max)
    rng = small.tile([BL, 1], f32, tag="rng")
    nc.gpsimd.tensor_tensor(rng[:], mx[:], mn[:], op=OP.subtract)
    rinv = small.tile([BL, 1], f32, tag="rinv")
    nc.vector.reciprocal(rinv[:], rng[:])
    out_sb = small.tile([BL, S], f32, tag="out_sb")
    nc.vector.tensor_scalar(
        out=out_sb[:], in0=tot16[:], scalar1=mn[:, 0:1], scalar2=rinv[:, 0:1],
        op0=OP.subtract, op1=OP.mult,
    )
    nc.sync.dma_start(out[:], out_sb[:])


def build_nc(path=None):
    nc = bacc.Bacc("TRN2", target_bir_lowering=False, debug=False)
    emb = nc.dram_tensor("emb", [BL, D, T], fp16, kind="ExternalInput")
    att = nc.dram_tensor("att", [BL * T, HS], fp16, kind="ExternalInput")
    fore = nc.dram_tensor("fore", [128, BL * DC], fp16, kind="ExternalInput")
    tgtT = nc.dram_tensor("tgtT", [128, NB], i32, kind="ExternalInput")
    out = nc.dram_tensor("out", [BL, S], f32, kind="ExternalOutput")
    with ExitStack() as ctx:
        tc_ = ctx.enter_context(tile.TileContext(nc))
        build_body(ctx, tc_, emb.ap(), att.ap(), fore.ap(), tgtT.ap(), out.ap())
    nc.compile()
    return nc


_NC_CACHE = {}


def get_nc(path=None):
    if "nc" not in _NC_CACHE:
        _NC_CACHE["nc"] = build_nc()
    return _NC_CACHE["nc"]


def make_in_maps(fore_rep_encoded, target_embed, align_attns, targets):
    LAYER_ID = 2
    att_l = np.transpose(np.asarray(align_attns[LAYER_ID]), (0, 2, 1, 3))  # [B,T,H,S]
    att_l = np.ascontiguousarray(att_l, dtype=np.float16)
    emb_d = np.ascontiguousarray(
        np.swapaxes(np.asarray(target_embed), 1, 2), dtype=np.float16
    )  # [B, D, T]
    fore_np = np.asarray(fore_rep_encoded, dtype=np.float16)  # [B, D]
    tgt_np = np.asarray(targets)[:, :T].astype(np.int32)      # [B, T]
    in_maps = []
    for cidx in range(NCORES):
        sl = slice(cidx * BL, (cidx + 1) * BL)
        fore_sl = fore_np[sl]                      # [BL, D]
        # fore cols [(p), (b, dc)]: col b*DC+dc = fore[b, dc*128:(dc+1)*128]
        fore_c = np.ascontiguousarray(
            fore_sl.reshape(BL, DC, 128).transpose(2, 0, 1).reshape(128, BL * DC)
        )
        tgt_sl = tgt_np[sl]                        # [BL, T]
        # tgtT [(p), (b, c)]: col b*TC+c = tgt[b, c*128:(c+1)*128]
        tgt_T = np.ascontiguousarray(
            tgt_sl.reshape(BL, TC, 128).transpose(2, 0, 1).reshape(128, NB)
        )
        in_maps.append(
            {
                "emb": np.ascontiguousarray(emb_d[sl]),
                "att": att_l[sl].reshape(BL * T, HS),
                "fore": fore_c,
                "tgtT": tgt_T,
            }
        )
    return in_maps


def kernel(fore_rep_encoded, target_embed, align_attns, targets):
    global LAST_EXEC_NS, LAST_RESULTS
    nc = get_nc()
    in_maps = make_in_maps(fore_rep_encoded, target_embed, align_attns, targets)
    trace = bool(os.environ.get("KERNEL_TRACE"))
    try:
        res = bass_utils.run_bass_kernel_spmd(
            nc, in_maps, core_ids=list(range(NCORES)), trace=trace
        )
    except ModuleNotFoundError:
        os.environ["BASS_NEVER_TRACE"] = "1"
        res = bass_utils.run_bass_kernel_spmd(
            nc, in_maps, core_ids=list(range(NCORES)), trace=False
        )
    LAST_EXEC_NS = res.exec_time_ns
    LAST_RESULTS = res
    return np.concatenate([r["out"] for r in res.results], axis=0)


# revision 11
# speedup vs baseline: 1.0807x; 1.0164x over previous
"""Trainium2 Bass kernel v2 for nn_CamAttnCon (topk-masked CAM attention).

Strategy (per core, 4 samples, D-layout fp16):
  - emb shipped host-transposed [BL, D, T] fp16; one big DMA per sample.
  - num[t] = e.f via ap-1 PE matmuls (lhsT = emb chunk [128D,128T], rhs = fore
    col) -> out [128,1] PSUM, nearly free in the cost model.
  - xn2[t] via tensor_scalar pow(e,2) (one DVE op per sample) + ap-1 ones
    matmuls.
  - w = num * rsqrt(xn2), masked; fp16 w16.  Selection threshold theta found
    by a 2-round 128-candidate parallel count search on a broadcast w (exact
    because round-2 step < fp16 ulp at theta).
  - Compaction by prefix-sum of the selection mask (triangular + ones PE
    matmuls), tie-robust; one-hot is_equal -> compact (row index, gather
    weight) pairs via PE; indirect-DMA gather of selected fp16 attn rows.
  - tot[s] = sum_j g_j att[t_j, h, s] over h via ap-1 matmuls in s-partition
    space; transpose back; min-max normalize.
"""

import os
import sys

sys.path.insert(0, "/opt/trn_rl_repo")

import numpy as np
from contextlib import ExitStack

import concourse.bass as bass
import concourse.bacc as bacc
import concourse.mybir as mybir
import concourse.tile as tile
from concourse.masks import make_identity
from concourse import bass_utils

f32 = mybir.dt.float32
fp16 = mybir.dt.float16
i32 = mybir.dt.int32
AX = mybir.AxisListType
OP = mybir.AluOpType
AF = mybir.ActivationFunctionType

B, T, D, H, S = 32, 512, 512, 8, 196
NCORES = 8
BL = B // NCORES            # 4 samples per core
TC = T // 128               # 4 t-chunks of 128
DC = D // 128               # 4 d-chunks of 128
HS = H * S                  # 1568
KK = 51                     # max top-k count
J = 52                      # padded slot count (>= KK, slots 1..52 hold pos)
NB = BL * TC                # 16 (b,tc) columns

# threshold search grid: theta in [LO0, LO0 + 128*STEP1)
LO0 = 0.0
STEP1 = 4.0 / 128.0         # 0.03125
STEP2 = STEP1 / 128.0       # 2.44e-4 < fp16 ulp at theta (~9.8e-4)

LAST_EXEC_NS = None
LAST_RESULTS = None


def build_body(ctx, tc_, emb, att, fore, tgtT, out):
    nc = tc_.nc

    const = ctx.enter_context(tc_.tile_pool(name="const", bufs=1))
    small = ctx.enter_context(tc_.tile_pool(name="small", bufs=1))
    embp = ctx.enter_context(tc_.tile_pool(name="embp", bufs=4))
    sqp = ctx.enter_context(tc_.tile_pool(name="sqp", bufs=2))
    wbcp = ctx.enter_context(tc_.tile_pool(name="wbcp", bufs=4))
    gatp = ctx.enter_context(tc_.tile_pool(name="gatp", bufs=4))

    ps_nx = ctx.enter_context(tc_.tile_pool(name="ps_nx", bufs=1, space="PSUM"))
    ps_wbc = ctx.enter_context(tc_.tile_pool(name="ps_wbc", bufs=2, space="PSUM"))
    ps_sm = ctx.enter_context(tc_.tile_pool(name="ps_sm", bufs=1, space="PSUM"))
    ps_tot = ctx.enter_context(tc_.tile_pool(name="ps_tot", bufs=1, space="PSUM"))

    # ---------------- input DMAs (tgt/fore first: tiny; then emb) -----------
    tgt_c = small.tile([128, NB], i32, tag="tgt_c")
    nc.sync.dma_start(tgt_c[:], tgtT[:])
    fore_c = small.tile([128, BL * DC], fp16, tag="fore_c")
    nc.scalar.dma_start(fore_c[:], fore[:])
    fore_c = small.tile([128, BL * DC], fp16, tag="fore_c")
    nc.sync.dma_start(fore_c[:], fore[:])
    embR = emb.rearrange("b (dc p) t -> b p dc t", p=128)
    embt = []
    for b in range(BL):
        e = embp.tile([128, DC * T], fp16, tag="emb")
        nc.sync.dma_start(e[:].rearrange("p (dc t) -> p dc t", dc=DC), embR[b])
        embt.append(e)

    # ---------------- constants (Pool/ACT, overlap the DMAs) ----------------
    # warm ACT function tables used later (rsqrt / reciprocal / copy)
    warm = const.tile([1, 1], f32, tag="warm")
    nc.vector.memset(warm[:], 1.0)
    warm2 = const.tile([1, 1], f32, tag="warm2")
    nc.scalar.sqrt(warm2[:], warm[:])
    nc.scalar.copy(warm2[:], warm[:])

    ones_col = const.tile([128, 1], fp16, tag="ones_col")
    nc.gpsimd.memset(ones_col[:], 1.0)
    ones_row = const.tile([1, 128], fp16, tag="ones_row")
    nc.gpsimd.memset(ones_row[:], 1.0)
    ones_row32 = const.tile([1, 128], f32, tag="ones_row32")
    nc.gpsimd.memset(ones_row32[:], 1.0)
    ones128 = const.tile([128, 128], fp16, tag="ones128")
    nc.gpsimd.memset(ones128[:], 1.0)
    zero_col = const.tile([128, 1], fp16, tag="zero_col")
    nc.gpsimd.memset(zero_col[:], 0.0)
    # bcsel4: slice tc [4, 128] has row tc all-ones (partition-bcast selector)
    bcsel4 = const.tile([BL, TC * 128], fp16, tag="bcsel4")
    nc.gpsimd.memset(bcsel4[:], 0.0)
    nc.gpsimd.affine_select(
        out=bcsel4[:].rearrange("p (blk j) -> p blk j", blk=TC),
        in_=bcsel4[:].rearrange("p (blk j) -> p blk j", blk=TC),
        compare_op=OP.not_equal,
        fill=1.0,
        base=0,
        pattern=[[-1, TC], [0, 128]],
        channel_multiplier=1,
    )

    # TRI[p, q] = 1 if p <= q  (inclusive prefix along partitions)
    qv_i = const.tile([128, 128], i32, tag="qv_i")
    nc.gpsimd.iota(qv_i[:], pattern=[[1, 128]], base=0, channel_multiplier=0)
    qv = const.tile([128, 128], f32, tag="qv")
    nc.gpsimd.tensor_copy(qv[:], qv_i[:])

    id16 = const.tile([128, 128], fp16, tag="id16")
    nc.gpsimd.memset(id16[:], 0.0)
    nc.gpsimd.affine_select(
        out=id16[:], in_=id16[:], compare_op=OP.not_equal, fill=1.0,
        base=0, pattern=[[1, 128]], channel_multiplier=-1,
    )
    id4 = const.tile([4, 4], f32, tag="id4")
    make_identity(nc, id4[:])

    # jvrep[p, (j, c)] = j + 1   (slot match values, c-packed for 2x mode)
    jvrep_i = const.tile([128, J * TC], i32, tag="jvrep_i")
    nc.gpsimd.iota(jvrep_i[:], pattern=[[1, J], [0, TC]], base=1, channel_multiplier=0)
    jvrep = const.tile([128, J * TC], fp16, tag="jvrep")
    nc.gpsimd.tensor_copy(jvrep[:], jvrep_i[:])

    # v2[p, (b, c, 0)] = global att row index b*T + c*128 + p ; col 1 <- g
    v2_i = const.tile([128, NB * 2], i32, tag="v2_i")
    nc.gpsimd.memset(v2_i[:], 0)
    nc.gpsimd.iota(
        v2_i[:].rearrange("p (b c two) -> p b c two", b=BL, c=TC)[:, :, :, 0],
        pattern=[[T, BL], [128, TC]], base=0, channel_multiplier=1,
    )
    v2 = const.tile([128, NB * 2], fp16, tag="v2")
    nc.gpsimd.tensor_copy(v2[:], v2_i[:])

    # tenrow[b, i] = 10*i  (for m = #{i<51 : 10i < seqlen})
    ten_i = const.tile([BL, KK], i32, tag="ten_i")
    nc.gpsimd.iota(ten_i[:], pattern=[[10, KK]], base=0, channel_multiplier=0)
    tenrow = const.tile([BL, KK], f32, tag="tenrow")
    nc.gpsimd.tensor_copy(tenrow[:], ten_i[:])

    # cand1[p] = LO0 + p*STEP1 ; iota2lo[p] = LO0 - STEP1 + p*STEP2
    pidx_i = const.tile([128, 1], i32, tag="pidx_i")
    nc.gpsimd.iota(pidx_i[:], pattern=[[0, 1]], base=0, channel_multiplier=1)
    pidx = const.tile([128, 1], f32, tag="pidx")
    nc.gpsimd.tensor_copy(pidx[:], pidx_i[:])
    tri = const.tile([128, 128], fp16, tag="tri")
    nc.gpsimd.tensor_scalar(
        out=tri[:], in0=qv[:], scalar1=pidx[:, 0:1], scalar2=None, op0=OP.is_ge
    )
    cand1 = const.tile([128, 1], f32, tag="cand1")
    nc.gpsimd.tensor_scalar(
        out=cand1[:], in0=pidx[:], scalar1=STEP1, scalar2=LO0, op0=OP.mult, op1=OP.add
    )
    iota2lo = const.tile([128, 1], f32, tag="iota2lo")
    nc.gpsimd.tensor_scalar(
        out=iota2lo[:], in0=pidx[:], scalar1=STEP2, scalar2=LO0 - STEP1,
        op0=OP.mult, op1=OP.add,
    )

    tgt_c = small.tile([128, NB], i32, tag="tgt_c")
    nc.gpsimd.dma_start(tgt_c[:], tgtT[:])
    # ---------------- mask / seqlen / m  (needs only tgt) -------------------
    m16 = small.tile([128, NB], fp16, tag="m16")
    nc.vector.tensor_scalar(
        out=m16[:], in0=tgt_c[:], scalar1=0, scalar2=None, op0=OP.is_gt
    )
    nc.gpsimd.memset(
        m16[0:1, :].rearrange("p (b c) -> p b c", c=TC)[:, :, 0:1], 1.0
    )
    seqrow_ps = ps_sm.tile([1, NB], f32, tag="sm")
    nc.tensor.matmul(out=seqrow_ps[:], lhsT=ones_col[:], rhs=m16[:], start=True, stop=True)
    seqb = small.tile([1, BL], f32, tag="seqb")
    nc.vector.tensor_reduce(
        seqb[:].unsqueeze(2),
        seqrow_ps[:].rearrange("p (b c) -> p b c", c=TC),
        axis=AX.X, op=OP.add,
    )
    seqcol_ps = ps_sm.tile([BL, 1], f32, tag="sm")
    nc.tensor.transpose(seqcol_ps[:], seqb[:], id4[0:1, 0:1])
    mcol = small.tile([BL, 1], f32, tag="mcol")
    mcnt = small.tile([BL, KK], f32, tag="mcnt")
    nc.vector.tensor_scalar(
        out=mcnt[:], in0=tenrow[:], scalar1=seqcol_ps[:, 0:1], scalar2=None,
        op0=OP.is_lt, op1=OP.add, accum_out=mcol[:],
    )
    mrow_ps = ps_sm.tile([1, BL], f32, tag="sm")
    nc.tensor.transpose(mrow_ps[:], mcol[:], id4[:])
    mrow = small.tile([1, BL], f32, tag="mrow")
    nc.scalar.copy(mrow[:], mrow_ps[:])
    mrow16 = small.tile([1, BL], fp16, tag="mrow16")
    nc.gpsimd.tensor_copy(mrow16[:], mrow[:])
    mbc_ps = ps_sm.tile([128, BL], f32, tag="sm")
    nc.tensor.matmul(out=mbc_ps[:], lhsT=ones_row[:], rhs=mrow16[:], start=True, stop=True)
    mbc = small.tile([128, BL], f32, tag="mbc")
    nc.scalar.copy(mbc[:], mbc_ps[:])

    # ---------------- num / xn2 via ap-1 matmuls ----------------------------
    nx_ps = ps_nx.tile([128, 3 * NB * DC], f32, tag="nx")
    num4_ps = nx_ps[:, 0 : NB * DC]
    xn24_ps = nx_ps[:, NB * DC : 2 * NB * DC]
    pos4_ps = nx_ps[:, 2 * NB * DC : 3 * NB * DC]
    sqt = []
    for b in range(BL):
        sq = sqp.tile([128, DC * T], fp16, tag="sq")
        nc.vector.tensor_tensor(sq[:], embt[b][:], embt[b][:], op=OP.mult)
        sqt.append(sq)
    for b in range(BL):
        eR = embt[b][:].rearrange("p (dc t) -> p dc t", dc=DC)
        sR = sqt[b][:].rearrange("p (dc t) -> p dc t", dc=DC)
        for tcc in range(TC):
            col = (b * TC + tcc) * DC
            for dc in range(DC):
                nc.tensor.matmul(
                    out=num4_ps[:, col + dc : col + dc + 1],
                    lhsT=eR[:, dc, tcc * 128 : (tcc + 1) * 128],
                    rhs=fore_c[:, b * DC + dc : b * DC + dc + 1],
                    start=True, stop=True,
                )
            for dc in range(DC):
                nc.tensor.matmul(
                    out=xn24_ps[:, col + dc : col + dc + 1],
                    lhsT=sR[:, dc, tcc * 128 : (tcc + 1) * 128],
                    rhs=ones_col[:],
                    start=True, stop=True,
                )
    num_sb = small.tile([128, NB], f32, tag="num_sb")
    nc.vector.tensor_reduce(
        num_sb[:].unsqueeze(2),
        num4_ps.rearrange("p (col dc) -> p col dc", dc=DC),
        axis=AX.X, op=OP.add,
    )
    xn2_sb = small.tile([128, NB], f32, tag="xn2_sb")
    nc.vector.tensor_reduce(
        xn2_sb[:].unsqueeze(2),
        xn24_ps.rearrange("p (col dc) -> p col dc", dc=DC),
        axis=AX.X, op=OP.add,
    )

    # ---------------- w columns -------------------------------------------
    xn = small.tile([128, NB], f32, tag="xn")
    nc.scalar.sqrt(xn[:], xn2_sb[:])
    rs = small.tile([128, NB], f32, tag="rs")
    nc.vector.reciprocal(rs[:], xn[:])
    w32 = small.tile([128, NB], f32, tag="w32")
    nc.vector.tensor_tensor(w32[:], num_sb[:], rs[:], op=OP.mult)
    w16 = small.tile([128, NB], fp16, tag="w16")
    nc.vector.tensor_tensor(w16[:], w32[:], m16[:], op=OP.mult)

    # ---------------- w rows + broadcast tiles -----------------------------
    wbc16 = []
    for b in range(BL):
        wrow_ps = ps_sm.tile([BL, 128], fp16, tag="sm")
        nc.tensor.transpose(wrow_ps[:], w16[:, b * TC : (b + 1) * TC], id16[:])
        wrow_b = small.tile([BL, 128], fp16, tag=f"wrow{b}")
        nc.scalar.copy(wrow_b[:], wrow_ps[:])
        wps = ps_wbc.tile([128, T], f32, tag="wbc")
        for tcc in range(TC):
            nc.tensor.matmul(
                out=wps[:, tcc * 128 : (tcc + 1) * 128],
                lhsT=bcsel4[:, tcc * 128 : (tcc + 1) * 128],
                rhs=wrow_b[:],
                start=True, stop=True,
            )
        wsb = wbcp.tile([128, T], fp16, tag="wbc16")
        nc.scalar.copy(wsb[:], wps[:])
        wbc16.append(wsb)

    # ---------------- 2-round threshold search -----------------------------
    cnt1 = small.tile([128, BL], f32, tag="cnt1")
    scr1 = wbcp.tile([128, T], fp16, tag="scr")
    for b in range(BL):
        nc.vector.tensor_scalar(
            out=scr1[:], in0=wbc16[b][:], scalar1=cand1[:, 0:1], scalar2=None,
            op0=OP.is_ge, op1=OP.add, accum_out=cnt1[:, b : b + 1],
        )
    selc1 = small.tile([128, BL], fp16, tag="selc1")
    nc.vector.tensor_tensor(selc1[:], cnt1[:], mbc[:], op=OP.is_ge)
    n1_ps = ps_sm.tile([1, BL], f32, tag="sm")
    nc.tensor.matmul(out=n1_ps[:], lhsT=ones_col[:], rhs=selc1[:], start=True, stop=True)
    n1row = small.tile([1, BL], fp16, tag="n1row")
    nc.scalar.copy(n1row[:], n1_ps[:])
    n1bc_ps = ps_sm.tile([128, BL], f32, tag="sm")
    nc.tensor.matmul(out=n1bc_ps[:], lhsT=ones_row[:], rhs=n1row[:], start=True, stop=True)
    cand2 = small.tile([128, BL], f32, tag="cand2")
    nc.vector.tensor_scalar(
        out=cand2[:], in0=n1bc_ps[:], scalar1=STEP1, scalar2=iota2lo[:, 0:1],
        op0=OP.mult, op1=OP.add,
    )
    cnt2 = small.tile([128, BL], f32, tag="cnt2")
    for b in range(BL):
        nc.vector.tensor_scalar(
            out=scr1[:], in0=wbc16[b][:], scalar1=cand2[:, b : b + 1], scalar2=None,
            op0=OP.is_ge, op1=OP.add, accum_out=cnt2[:, b : b + 1],
        )
    selc2 = small.tile([128, BL], fp16, tag="selc2")
    nc.vector.tensor_tensor(selc2[:], cnt2[:], mbc[:], op=OP.is_ge)
    n2_ps = ps_sm.tile([1, BL], f32, tag="sm")
    nc.tensor.matmul(out=n2_ps[:], lhsT=ones_col[:], rhs=selc2[:], start=True, stop=True)
    # theta = LO0 + (n1-1)*STEP1 + (n2-1)*STEP2
    t1 = small.tile([1, BL], f32, tag="t1")
    nc.vector.tensor_scalar(
        out=t1[:], in0=n2_ps[:], scalar1=STEP2, scalar2=LO0 - STEP1 - STEP2,
        op0=OP.mult, op1=OP.add,
    )
    theta = small.tile([1, BL], f32, tag="theta")
    nc.vector.scalar_tensor_tensor(
        out=theta[:], in0=n1row[:], scalar=STEP1, in1=t1[:], op0=OP.mult, op1=OP.add
    )
    thbc_ps = ps_sm.tile([128, BL], f32, tag="sm")
    nc.tensor.matmul(out=thbc_ps[:], lhsT=ones_row32[:], rhs=theta[:], start=True, stop=True)

    # ---------------- selection, prefix, one-hot compaction -----------------
    sel16 = small.tile([128, NB], fp16, tag="sel16")
    nc.vector.tensor_tensor(
        sel16[:].rearrange("p (b c) -> p b c", c=TC),
        w16[:].rearrange("p (b c) -> p b c", c=TC),
        thbc_ps[:].unsqueeze(2).broadcast_to([128, BL, TC]),
        op=OP.is_ge,
    )
    g16 = small.tile([128, NB], fp16, tag="g16")
    nc.vector.scalar_tensor_tensor(
        out=g16[:], in0=w16[:], scalar=0.0, in1=sel16[:], op0=OP.max, op1=OP.mult
    )
    nc.gpsimd.tensor_copy(
        v2[:].rearrange("p (b c two) -> p b c two", b=BL, c=TC)[:, :, :, 1],
        g16[:].rearrange("p (b c) -> p b c", c=TC),
    )

    # pos[q, (b,c)] = prefix count of sel up to global position (c*128+q)
    # pos4 col ((b,c), k): k=0 -> TRI@sel[c]; k>=1 -> ONES@sel[c-k] (zero pad)
    selR = sel16[:].rearrange("p (b c) -> p b c", c=TC)
    pos4R = pos4_ps.rearrange("p (col k) -> p col k", k=DC)
    for b in range(BL):
        for c in range(TC):
            nc.tensor.matmul(
                out=pos4R[:, b * TC + c, 0:1],
                lhsT=tri[:], rhs=selR[:, b, c].unsqueeze(1),
                start=True, stop=True,
            )
    for b in range(BL):
        for c in range(TC):
            for k in range(1, DC):
                rhs = (
                    selR[:, b, c - k].unsqueeze(1) if c - k >= 0 else zero_col[:]
                )
                nc.tensor.matmul(
                    out=pos4R[:, b * TC + c, k : k + 1],
                    lhsT=ones128[:], rhs=rhs,
                    start=True, stop=True,
                )
    pos_sb = small.tile([128, NB], f32, tag="pos_sb")
    nc.vector.tensor_reduce(
        pos_sb[:].unsqueeze(2),
        pos4R,
        axis=AX.X, op=OP.add,
    )
    posm = small.tile([128, NB], fp16, tag="posm")
    nc.vector.tensor_tensor(posm[:], pos_sb[:], sel16[:], op=OP.mult)

    # st[p, (b, j, c)] = [posm[p, (b,c)] == j+1]
    st4 = small.tile([128, BL * J * TC], fp16, tag="st4")
    nc.vector.tensor_tensor(
        out=st4[:].rearrange("p (b j c) -> p b j c", b=BL, j=J),
        in0=posm[:].rearrange("p (b c) -> p b c", c=TC).unsqueeze(2).broadcast_to([128, BL, J, TC]),
        in1=jvrep[:].rearrange("p (j c) -> p j c", j=J).unsqueeze(1).broadcast_to([128, BL, J, TC]),
        op=OP.is_equal,
    )
    st4R = st4[:].rearrange("p (b j c) -> p b j c", b=BL, j=J)
    v2R = v2[:].rearrange("p (b c two) -> p b c two", b=BL, c=TC)
    staks = []
    for b in range(BL):
        stak_ps = ps_sm.tile([2, J], f32, tag="sm")
        for c in range(TC):
            nc.tensor.matmul(
                out=stak_ps[:],
                lhsT=v2R[:, b, c, :],
                rhs=st4R[:, b, :, c],
                start=(c == 0), stop=(c == TC - 1),
            )
        stak_b = small.tile([2, J], f32, tag=f"stak{b}")
        nc.scalar.copy(stak_b[:], stak_ps[:])
        staks.append(stak_b)

    # ---------------- per-sample gather + tot ------------------------------
    gsel = small.tile([J, BL], fp16, tag="gsel")
    nc.gpsimd.memset(gsel[:], 0.0)
    totAB_ps = ps_tot.tile([128, 2 * BL * H], f32, tag="totAB")
    totA_ps = totAB_ps[:, 0 : BL * H]
    totB_ps = totAB_ps[0 : S - 128, BL * H : 2 * BL * H]
    for b in range(BL):
        pst_ps = ps_sm.tile([J, 2], f32, tag="sm")
        nc.tensor.transpose(pst_ps[:], staks[b][:], id4[0:2, 0:2])
        idx_b = small.tile([J, 1], i32, tag=f"idx{b}")
        nc.scalar.copy(idx_b[:], pst_ps[:, 0:1])
        nc.scalar.copy(gselb[b][:, b : b + 1], pst_ps[:, 1:2])
        gat_b = gatp.tile([J, HS], fp16, tag="gat")
        nc.gpsimd.indirect_dma_start(
            out=gat_b[:],
            out_offset=None,
            in_=att[:],
            in_offset=bass.IndirectOffsetOnAxis(ap=idx_b[:, 0:1], axis=0),
        )
        for h in range(H):
            nc.tensor.matmul(
                out=totA_ps[:, b * H + h : b * H + h + 1],
                lhsT=gat_b[:, h * S : h * S + 128],
                rhs=gsel[:, b : b + 1],
                start=True, stop=True,
            )
        for h in range(H):
            nc.tensor.matmul(
                out=totB_ps[:, b * H + h : b * H + h + 1],
                lhsT=gat_b[:, h * S + 128 : (h + 1) * S],
                rhs=gsel[:, b : b + 1],
                start=True, stop=True,
            )

    # ---------------- transpose back + normalize ---------------------------
    totA32 = small.tile([128, BL], f32, tag="totA32")
    nc.vector.tensor_reduce(
        totA32[:].unsqueeze(2),
        totA_ps.rearrange("p (b h) -> p b h", h=H),
        axis=AX.X, op=OP.add,
    )
    totB32 = small.tile([S - 128, BL], f32, tag="totB32")
    nc.vector.tensor_reduce(
        totB32[:].unsqueeze(2),
        totB_ps.rearrange("p (b h) -> p b h", h=H),
        axis=AX.X, op=OP.add,
    )
    totA16 = small.tile([128, BL], fp16, tag="totA16")
    nc.scalar.copy(totA16[:], totA32[:])
    totB16 = small.tile([S - 128, BL], fp16, tag="totB16")
    nc.scalar.copy(totB16[:], totB32[:])
    trAB_ps = ps_tot.tile([BL, S], fp16, tag="trAB")
    nc.tensor.transpose(trAB_ps[:, 0:128], totA16[:], id16[:])
    nc.tensor.transpose(trAB_ps[:, 128:S], totB16[:], id16[0 : S - 128, 0 : S - 128])
    tot16 = small.tile([BL, S], fp16, tag="tot16")
    nc.scalar.copy(tot16[:], trAB_ps[:])

    mn = small.tile([BL, 1], f32, tag="mn")
    nc.vector.tensor_reduce(mn[:], tot16[:], axis=AX.X, op=OP.min)
    mx = small.tile([BL, 1], f32, tag="mx")
    nc.vector.tensor_reduce(mx[:], tot16[:], axis=AX.X, op=OP.max)
    rng = small.tile([BL, 1], f32, tag="rng")
    nc.gpsimd.tensor_tensor(rng[:], mx[:], mn[:], op=OP.subtract)
    rinv = small.tile([BL, 1], f32, tag="rinv")
    nc.vector.reciprocal(rinv[:], rng[:])
    out_sb = small.tile([BL, S], f32, tag="out_sb")
    nc.vector.tensor_scalar(
        out=out_sb[:], in0=tot16[:], scalar1=mn[:, 0:1], scalar2=rinv[:, 0:1],
        op0=OP.subtract, op1=OP.mult,
    )
    nc.sync.dma_start(out[:], out_sb[:])


def build_nc(path=None):
    nc = bacc.Bacc("TRN2", target_bir_lowering=False, debug=False)
    emb = nc.dram_tensor("emb", [BL, D, T], fp16, kind="ExternalInput")
    att = nc.dram_tensor("att", [BL * T, HS], fp16, kind="ExternalInput")
    fore = nc.dram_tensor("fore", [128, BL * DC], fp16, kind="ExternalInput")
    tgtT = nc.dram_tensor("tgtT", [128, NB], i32, kind="ExternalInput")
    out = nc.dram_tensor("out", [BL, S], f32, kind="ExternalOutput")
    with ExitStack() as ctx:
        tc_ = ctx.enter_context(tile.TileContext(nc))
        build_body(ctx, tc_, emb.ap(), att.ap(), fore.ap(), tgtT.ap(), out.ap())
    nc.compile()
    return nc


_NC_CACHE = {}


def get_nc(path=None):
    if "nc" not in _NC_CACHE:
        _NC_CACHE["nc"] = build_nc()
    return _NC_CACHE["nc"]


def make_in_maps(fore_rep_encoded, target_embed, align_attns, targets):
    LAYER_ID = 2
    att_l = np.transpose(np.asarray(align_attns[LAYER_ID]), (0, 2, 1, 3))  # [B,T,H,S]
    att_l = np.ascontiguousarray(att_l, dtype=np.float16)
    emb_d = np.ascontiguousarray(
        np.swapaxes(np.asarray(target_embed), 1, 2), dtype=np.float16
    )  # [B, D, T]
    fore_np = np.asarray(fore_rep_encoded, dtype=np.float16)  # [B, D]
    tgt_np = np.asarray(targets)[:, :T].astype(np.int32)      # [B, T]
    in_maps = []
    for cidx in range(NCORES):
        sl = slice(cidx * BL, (cidx + 1) * BL)
        fore_sl = fore_np[sl]                      # [BL, D]
        # fore cols [(p), (b, dc)]: col b*DC+dc = fore[b, dc*128:(dc+1)*128]
        fore_c = np.ascontiguousarray(
            fore_sl.reshape(BL, DC, 128).transpose(2, 0, 1).reshape(128, BL * DC)
        )
        tgt_sl = tgt_np[sl]                        # [BL, T]
        # tgtT [(p), (b, c)]: col b*TC+c = tgt[b, c*128:(c+1)*128]
        tgt_T = np.ascontiguousarray(
            tgt_sl.reshape(BL, TC, 128).transpose(2, 0, 1).reshape(128, NB)
        )
        in_maps.append(
            {
                "emb": np.ascontiguousarray(emb_d[sl]),
                "att": att_l[sl].reshape(BL * T, HS),
                "fore": fore_c,
                "tgtT": tgt_T,
            }
        )
    return in_maps


def kernel(fore_rep_encoded, target_embed, align_attns, targets):
    global LAST_EXEC_NS, LAST_RESULTS
    nc = get_nc()
    in_maps = make_in_maps(fore_rep_encoded, target_embed, align_attns, targets)
    trace = bool(os.environ.get("KERNEL_TRACE"))
    try:
        res = bass_utils.run_bass_kernel_spmd(
            nc, in_maps, core_ids=list(range(NCORES)), trace=trace
        )
    except ModuleNotFoundError:
        os.environ["BASS_NEVER_TRACE"] = "1"
        res = bass_utils.run_bass_kernel_spmd(
            nc, in_maps, core_ids=list(range(NCORES)), trace=False
        )
    LAST_EXEC_NS = res.exec_time_ns
    LAST_RESULTS = res
    return np.concatenate([r["out"] for r in res.results], axis=0)


# revision 12
# speedup vs baseline: 1.1172x; 1.0338x over previous
"""Trainium2 Bass kernel v2 for nn_CamAttnCon (topk-masked CAM attention).

Strategy (per core, 4 samples, D-layout fp16):
  - emb shipped host-transposed [BL, D, T] fp16; one big DMA per sample.
  - num[t] = e.f via ap-1 PE matmuls (lhsT = emb chunk [128D,128T], rhs = fore
    col) -> out [128,1] PSUM, nearly free in the cost model.
  - xn2[t] via tensor_scalar pow(e,2) (one DVE op per sample) + ap-1 ones
    matmuls.
  - w = num * rsqrt(xn2), masked; fp16 w16.  Selection threshold theta found
    by a 2-round 128-candidate parallel count search on a broadcast w (exact
    because round-2 step < fp16 ulp at theta).
  - Compaction by prefix-sum of the selection mask (triangular + ones PE
    matmuls), tie-robust; one-hot is_equal -> compact (row index, gather
    weight) pairs via PE; indirect-DMA gather of selected fp16 attn rows.
  - tot[s] = sum_j g_j att[t_j, h, s] over h via ap-1 matmuls in s-partition
    space; transpose back; min-max normalize.
"""

import os
import sys

sys.path.insert(0, "/opt/trn_rl_repo")

import numpy as np
from contextlib import ExitStack

import concourse.bass as bass
import concourse.bacc as bacc
import concourse.mybir as mybir
import concourse.tile as tile
from concourse.masks import make_identity
from concourse import bass_utils

f32 = mybir.dt.float32
fp16 = mybir.dt.float16
i32 = mybir.dt.int32
AX = mybir.AxisListType
OP = mybir.AluOpType
AF = mybir.ActivationFunctionType

B, T, D, H, S = 32, 512, 512, 8, 196
NCORES = 8
BL = B // NCORES            # 4 samples per core
TC = T // 128               # 4 t-chunks of 128
DC = D // 128               # 4 d-chunks of 128
HS = H * S                  # 1568
KK = 51                     # max top-k count
J = 52                      # padded slot count (>= KK, slots 1..52 hold pos)
NB = BL * TC                # 16 (b,tc) columns

# threshold search grid: theta in [LO0, LO0 + 128*STEP1)
LO0 = 0.0
STEP1 = 4.0 / 128.0         # 0.03125
STEP2 = STEP1 / 128.0       # 2.44e-4 < fp16 ulp at theta (~9.8e-4)

LAST_EXEC_NS = None
LAST_RESULTS = None


def build_body(ctx, tc_, emb, att, fore, tgtT, out):
    nc = tc_.nc

    const = ctx.enter_context(tc_.tile_pool(name="const", bufs=1))
    small = ctx.enter_context(tc_.tile_pool(name="small", bufs=1))
    embp = ctx.enter_context(tc_.tile_pool(name="embp", bufs=4))
    sqp = ctx.enter_context(tc_.tile_pool(name="sqp", bufs=2))
    wbcp = ctx.enter_context(tc_.tile_pool(name="wbcp", bufs=4))
    gatp = ctx.enter_context(tc_.tile_pool(name="gatp", bufs=4))

    ps_nx = ctx.enter_context(tc_.tile_pool(name="ps_nx", bufs=1, space="PSUM"))
    ps_wbc = ctx.enter_context(tc_.tile_pool(name="ps_wbc", bufs=1, space="PSUM"))
    ps_sm = ctx.enter_context(tc_.tile_pool(name="ps_sm", bufs=2, space="PSUM"))
    ps_tot = ctx.enter_context(tc_.tile_pool(name="ps_tot", bufs=1, space="PSUM"))

    # ---------------- input DMAs (tgt/fore first: tiny; then emb) -----------
    tgt_c = small.tile([128, NB], i32, tag="tgt_c")
    nc.sync.dma_start(tgt_c[:], tgtT[:])
    fore_c = small.tile([128, BL * DC], fp16, tag="fore_c")
    nc.scalar.dma_start(fore_c[:], fore[:])
    fore_c = small.tile([128, BL * DC], fp16, tag="fore_c")
    nc.sync.dma_start(fore_c[:], fore[:])
    embR = emb.rearrange("b (dc p) t -> b p dc t", p=128)
    embt = []
    for b in range(BL):
        e = embp.tile([128, DC * T], fp16, tag="emb")
        nc.sync.dma_start(e[:].rearrange("p (dc t) -> p dc t", dc=DC), embR[b])
        embt.append(e)

    # ---------------- constants (Pool/ACT, overlap the DMAs) ----------------
    # warm ACT function tables used later (rsqrt / reciprocal / copy)
    warm = const.tile([1, 1], f32, tag="warm")
    nc.vector.memset(warm[:], 1.0)
    warm2 = const.tile([1, 1], f32, tag="warm2")
    nc.scalar.sqrt(warm2[:], warm[:])
    nc.scalar.copy(warm2[:], warm[:])

    ones_col = const.tile([128, 1], fp16, tag="ones_col")
    nc.gpsimd.memset(ones_col[:], 1.0)
    ones_row = const.tile([1, 128], fp16, tag="ones_row")
    nc.gpsimd.memset(ones_row[:], 1.0)
    ones_row32 = const.tile([1, 128], f32, tag="ones_row32")
    nc.gpsimd.memset(ones_row32[:], 1.0)
    ones128 = const.tile([128, 128], fp16, tag="ones128")
    nc.gpsimd.memset(ones128[:], 1.0)
    zero_col = const.tile([128, 1], fp16, tag="zero_col")
    nc.gpsimd.memset(zero_col[:], 0.0)
    # bcsel4: slice tc [4, 128] has row tc all-ones (partition-bcast selector)
    bcsel4 = const.tile([BL, TC * 128], fp16, tag="bcsel4")
    nc.gpsimd.memset(bcsel4[:], 0.0)
    nc.gpsimd.affine_select(
        out=bcsel4[:].rearrange("p (blk j) -> p blk j", blk=TC),
        in_=bcsel4[:].rearrange("p (blk j) -> p blk j", blk=TC),
        compare_op=OP.not_equal,
        fill=1.0,
        base=0,
        pattern=[[-1, TC], [0, 128]],
        channel_multiplier=1,
    )

    # TRI[p, q] = 1 if p <= q  (inclusive prefix along partitions)
    qv_i = const.tile([128, 128], i32, tag="qv_i")
    nc.gpsimd.iota(qv_i[:], pattern=[[1, 128]], base=0, channel_multiplier=0)
    qv = const.tile([128, 128], f32, tag="qv")
    nc.gpsimd.tensor_copy(qv[:], qv_i[:])

    id16 = const.tile([128, 128], fp16, tag="id16")
    nc.gpsimd.memset(id16[:], 0.0)
    nc.gpsimd.affine_select(
        out=id16[:], in_=id16[:], compare_op=OP.not_equal, fill=1.0,
        base=0, pattern=[[1, 128]], channel_multiplier=-1,
    )
    id4 = const.tile([4, 4], f32, tag="id4")
    make_identity(nc, id4[:])

    # jvrep[p, (j, c)] = j + 1   (slot match values, c-packed for 2x mode)
    jvrep_i = const.tile([128, J * TC], i32, tag="jvrep_i")
    nc.gpsimd.iota(jvrep_i[:], pattern=[[1, J], [0, TC]], base=1, channel_multiplier=0)
    jvrep = const.tile([128, J * TC], fp16, tag="jvrep")
    nc.gpsimd.tensor_copy(jvrep[:], jvrep_i[:])

    # v2[p, (b, c, 0)] = global att row index b*T + c*128 + p ; col 1 <- g
    v2_i = const.tile([128, NB * 2], i32, tag="v2_i")
    nc.gpsimd.memset(v2_i[:], 0)
    nc.gpsimd.iota(
        v2_i[:].rearrange("p (b c two) -> p b c two", b=BL, c=TC)[:, :, :, 0],
        pattern=[[T, BL], [128, TC]], base=0, channel_multiplier=1,
    )
    v2 = const.tile([128, NB * 2], fp16, tag="v2")
    nc.gpsimd.tensor_copy(v2[:], v2_i[:])

    # tenrow[b, i] = 10*i  (for m = #{i<51 : 10i < seqlen})
    ten_i = const.tile([BL, KK], i32, tag="ten_i")
    nc.gpsimd.iota(ten_i[:], pattern=[[10, KK]], base=0, channel_multiplier=0)
    tenrow = const.tile([BL, KK], f32, tag="tenrow")
    nc.gpsimd.tensor_copy(tenrow[:], ten_i[:])

    # cand1[p] = LO0 + p*STEP1 ; iota2lo[p] = LO0 - STEP1 + p*STEP2
    pidx_i = const.tile([128, 1], i32, tag="pidx_i")
    nc.gpsimd.iota(pidx_i[:], pattern=[[0, 1]], base=0, channel_multiplier=1)
    pidx = const.tile([128, 1], f32, tag="pidx")
    nc.gpsimd.tensor_copy(pidx[:], pidx_i[:])
    tri = const.tile([128, 128], fp16, tag="tri")
    nc.gpsimd.tensor_scalar(
        out=tri[:], in0=qv[:], scalar1=pidx[:, 0:1], scalar2=None, op0=OP.is_ge
    )
    cand1 = const.tile([128, 1], f32, tag="cand1")
    nc.gpsimd.tensor_scalar(
        out=cand1[:], in0=pidx[:], scalar1=STEP1, scalar2=LO0, op0=OP.mult, op1=OP.add
    )
    iota2lo = const.tile([128, 1], f32, tag="iota2lo")
    nc.gpsimd.tensor_scalar(
        out=iota2lo[:], in0=pidx[:], scalar1=STEP2, scalar2=LO0 - STEP1,
        op0=OP.mult, op1=OP.add,
    )

    tgt_c = small.tile([128, NB], i32, tag="tgt_c")
    nc.gpsimd.dma_start(tgt_c[:], tgtT[:])
    # ---------------- mask / seqlen / m  (needs only tgt) -------------------
    m16 = small.tile([128, NB], fp16, tag="m16")
    nc.vector.tensor_scalar(
        out=m16[:], in0=tgt_c[:], scalar1=0, scalar2=None, op0=OP.is_gt
    )
    nc.gpsimd.memset(
        m16[0:1, :].rearrange("p (b c) -> p b c", c=TC)[:, :, 0:1], 1.0
    )
    seqrow_ps = ps_sm.tile([1, NB], f32, tag="sm")
    nc.tensor.matmul(out=seqrow_ps[:], lhsT=ones_col[:], rhs=m16[:], start=True, stop=True)
    seqb = small.tile([1, BL], f32, tag="seqb")
    nc.vector.tensor_reduce(
        seqb[:].unsqueeze(2),
        seqrow_ps[:].rearrange("p (b c) -> p b c", c=TC),
        axis=AX.X, op=OP.add,
    )
    seqcol_ps = ps_sm.tile([BL, 1], f32, tag="sm")
    nc.tensor.transpose(seqcol_ps[:], seqb[:], id4[0:1, 0:1])
    mcol = small.tile([BL, 1], f32, tag="mcol")
    mcnt = small.tile([BL, KK], f32, tag="mcnt")
    nc.vector.tensor_scalar(
        out=mcnt[:], in0=tenrow[:], scalar1=seqcol_ps[:, 0:1], scalar2=None,
        op0=OP.is_lt, op1=OP.add, accum_out=mcol[:],
    )
    mrow_ps = ps_sm.tile([1, BL], f32, tag="sm")
    nc.tensor.transpose(mrow_ps[:], mcol[:], id4[:])
    mrow = small.tile([1, BL], f32, tag="mrow")
    nc.scalar.copy(mrow[:], mrow_ps[:])
    mrow16 = small.tile([1, BL], fp16, tag="mrow16")
    nc.gpsimd.tensor_copy(mrow16[:], mrow[:])
    mbc_ps = ps_sm.tile([128, BL], f32, tag="sm")
    nc.tensor.matmul(out=mbc_ps[:], lhsT=ones_row[:], rhs=mrow16[:], start=True, stop=True)
    mbc = small.tile([128, BL], f32, tag="mbc")
    nc.scalar.copy(mbc[:], mbc_ps[:])

    # ---------------- num / xn2 via ap-1 matmuls ----------------------------
    nx_ps = ps_nx.tile([128, 3 * NB * DC], f32, tag="nx")
    num4_ps = nx_ps[:, 0 : NB * DC]
    xn24_ps = nx_ps[:, NB * DC : 2 * NB * DC]
    pos4_ps = nx_ps[:, 2 * NB * DC : 3 * NB * DC]
    sqt = []
    for b in range(BL):
        sq = sqp.tile([128, DC * T], fp16, tag="sq")
        nc.vector.tensor_tensor(sq[:], embt[b][:], embt[b][:], op=OP.mult)
        sqt.append(sq)
    for b in range(BL):
        eR = embt[b][:].rearrange("p (dc t) -> p dc t", dc=DC)
        sR = sqt[b][:].rearrange("p (dc t) -> p dc t", dc=DC)
        for tcc in range(TC):
            col = (b * TC + tcc) * DC
            for dc in range(DC):
                nc.tensor.matmul(
                    out=num4_ps[:, col + dc : col + dc + 1],
                    lhsT=eR[:, dc, tcc * 128 : (tcc + 1) * 128],
                    rhs=fore_c[:, b * DC + dc : b * DC + dc + 1],
                    start=True, stop=True,
                )
            for dc in range(DC):
                nc.tensor.matmul(
                    out=xn24_ps[:, col + dc : col + dc + 1],
                    lhsT=sR[:, dc, tcc * 128 : (tcc + 1) * 128],
                    rhs=ones_col[:],
                    start=True, stop=True,
                )
    num_sb = small.tile([128, NB], f32, tag="num_sb")
    nc.vector.tensor_reduce(
        num_sb[:].unsqueeze(2),
        num4_ps.rearrange("p (col dc) -> p col dc", dc=DC),
        axis=AX.X, op=OP.add,
    )
    xn2_sb = small.tile([128, NB], f32, tag="xn2_sb")
    nc.vector.tensor_reduce(
        xn2_sb[:].unsqueeze(2),
        xn24_ps.rearrange("p (col dc) -> p col dc", dc=DC),
        axis=AX.X, op=OP.add,
    )

    # ---------------- w columns -------------------------------------------
    xn = small.tile([128, NB], f32, tag="xn")
    nc.scalar.sqrt(xn[:], xn2_sb[:])
    rs = small.tile([128, NB], f32, tag="rs")
    nc.vector.reciprocal(rs[:], xn[:])
    w32 = small.tile([128, NB], f32, tag="w32")
    nc.vector.tensor_tensor(w32[:], num_sb[:], rs[:], op=OP.mult)
    w16 = small.tile([128, NB], fp16, tag="w16")
    nc.vector.tensor_tensor(w16[:], w32[:], m16[:], op=OP.mult)

    # ---------------- w rows + broadcast tiles -----------------------------
    wbc16 = []
    for b in range(BL):
        wrow_ps = ps_sm.tile([BL, 128], fp16, tag="sm")
        nc.tensor.transpose(wrow_ps[:], w16[:, b * TC : (b + 1) * TC], id16[:])
        wrow_b = small.tile([BL, 128], fp16, tag=f"wrow{b}")
        nc.scalar.copy(wrow_b[:], wrow_ps[:])
        wps = ps_wbc.tile([128, T], f32, tag="wbc")
        for tcc in range(TC):
            nc.tensor.matmul(
                out=wps[:, tcc * 128 : (tcc + 1) * 128],
                lhsT=bcsel4[:, tcc * 128 : (tcc + 1) * 128],
                rhs=wrow_b[:],
                start=True, stop=True,
            )
        wsb = wbcp.tile([128, T], fp16, tag="wbc16")
        nc.scalar.copy(wsb[:], wps[:])
        wbc16.append(wsb)

    # ---------------- 2-round threshold search -----------------------------
    cnt1 = small.tile([128, BL], f32, tag="cnt1")
    scr1 = wbcp.tile([128, T], fp16, tag="scr")
    for b in range(BL):
        nc.vector.tensor_scalar(
            out=scr1[:], in0=wbc16[b][:], scalar1=cand1[:, 0:1], scalar2=None,
            op0=OP.is_ge, op1=OP.add, accum_out=cnt1[:, b : b + 1],
        )
    selc1 = small.tile([128, BL], fp16, tag="selc1")
    nc.vector.tensor_tensor(selc1[:], cnt1[:], mbc[:], op=OP.is_ge)
    n1_ps = ps_sm.tile([1, BL], f32, tag="sm")
    nc.tensor.matmul(out=n1_ps[:], lhsT=ones_col[:], rhs=selc1[:], start=True, stop=True)
    n1row = small.tile([1, BL], fp16, tag="n1row")
    nc.scalar.copy(n1row[:], n1_ps[:])
    n1bc_ps = ps_sm.tile([128, BL], f32, tag="sm")
    nc.tensor.matmul(out=n1bc_ps[:], lhsT=ones_row[:], rhs=n1row[:], start=True, stop=True)
    cand2 = small.tile([128, BL], f32, tag="cand2")
    nc.vector.tensor_scalar(
        out=cand2[:], in0=n1bc_ps[:], scalar1=STEP1, scalar2=iota2lo[:, 0:1],
        op0=OP.mult, op1=OP.add,
    )
    cnt2 = small.tile([128, BL], f32, tag="cnt2")
    for b in range(BL):
        nc.vector.tensor_scalar(
            out=scr1[:], in0=wbc16[b][:], scalar1=cand2[:, b : b + 1], scalar2=None,
            op0=OP.is_ge, op1=OP.add, accum_out=cnt2[:, b : b + 1],
        )
    selc2 = small.tile([128, BL], fp16, tag="selc2")
    nc.vector.tensor_tensor(selc2[:], cnt2[:], mbc[:], op=OP.is_ge)
    n2_ps = ps_sm.tile([1, BL], f32, tag="sm")
    nc.tensor.matmul(out=n2_ps[:], lhsT=ones_col[:], rhs=selc2[:], start=True, stop=True)
    # theta = LO0 + (n1-1)*STEP1 + (n2-1)*STEP2
    t1 = small.tile([1, BL], f32, tag="t1")
    nc.vector.tensor_scalar(
        out=t1[:], in0=n2_ps[:], scalar1=STEP2, scalar2=LO0 - STEP1 - STEP2,
        op0=OP.mult, op1=OP.add,
    )
    theta = small.tile([1, BL], f32, tag="theta")
    nc.vector.scalar_tensor_tensor(
        out=theta[:], in0=n1row[:], scalar=STEP1, in1=t1[:], op0=OP.mult, op1=OP.add
    )
    thbc_ps = ps_sm.tile([128, BL], f32, tag="sm")
    nc.tensor.matmul(out=thbc_ps[:], lhsT=ones_row32[:], rhs=theta[:], start=True, stop=True)

    # ---------------- selection, prefix, one-hot compaction -----------------
    sel16 = small.tile([128, NB], fp16, tag="sel16")
    nc.vector.tensor_tensor(
        sel16[:].rearrange("p (b c) -> p b c", c=TC),
        w16[:].rearrange("p (b c) -> p b c", c=TC),
        thbc_ps[:].unsqueeze(2).broadcast_to([128, BL, TC]),
        op=OP.is_ge,
    )
    g16 = small.tile([128, NB], fp16, tag="g16")
    nc.vector.scalar_tensor_tensor(
        out=g16[:], in0=w16[:], scalar=0.0, in1=sel16[:], op0=OP.max, op1=OP.mult
    )
    nc.gpsimd.tensor_copy(
        v2[:].rearrange("p (b c two) -> p b c two", b=BL, c=TC)[:, :, :, 1],
        g16[:].rearrange("p (b c) -> p b c", c=TC),
    )

    # pos[q, (b,c)] = prefix count of sel up to global position (c*128+q)
    # pos4 col ((b,c), k): k=0 -> TRI@sel[c]; k>=1 -> ONES@sel[c-k] (zero pad)
    selR = sel16[:].rearrange("p (b c) -> p b c", c=TC)
    pos4R = pos4_ps.rearrange("p (col k) -> p col k", k=DC)
    for b in range(BL):
        for c in range(TC):
            nc.tensor.matmul(
                out=pos4R[:, b * TC + c, 0:1],
                lhsT=tri[:], rhs=selR[:, b, c].unsqueeze(1),
                start=True, stop=True,
            )
    for b in range(BL):
        for c in range(TC):
            for k in range(1, DC):
                rhs = (
                    selR[:, b, c - k].unsqueeze(1) if c - k >= 0 else zero_col[:]
                )
                nc.tensor.matmul(
                    out=pos4R[:, b * TC + c, k : k + 1],
                    lhsT=ones128[:], rhs=rhs,
                    start=True, stop=True,
                )
    pos_sb = small.tile([128, NB], f32, tag="pos_sb")
    nc.vector.tensor_reduce(
        pos_sb[:].unsqueeze(2),
        pos4R,
        axis=AX.X, op=OP.add,
    )
    posm = small.tile([128, NB], fp16, tag="posm")
    nc.vector.tensor_tensor(posm[:], pos_sb[:], sel16[:], op=OP.mult)

    # st[p, (b, j, c)] = [posm[p, (b,c)] == j+1]
    st4 = small.tile([128, BL * J * TC], fp16, tag="st4")
    nc.vector.tensor_tensor(
        out=st4[:].rearrange("p (b j c) -> p b j c", b=BL, j=J),
        in0=posm[:].rearrange("p (b c) -> p b c", c=TC).unsqueeze(2).broadcast_to([128, BL, J, TC]),
        in1=jvrep[:].rearrange("p (j c) -> p j c", j=J).unsqueeze(1).broadcast_to([128, BL, J, TC]),
        op=OP.is_equal,
    )
    st4R = st4[:].rearrange("p (b j c) -> p b j c", b=BL, j=J)
    v2R = v2[:].rearrange("p (b c two) -> p b c two", b=BL, c=TC)
    staks = []
    for b in range(BL):
        stak_ps = ps_sm.tile([2, J], f32, tag="sm")
        for c in range(TC):
            nc.tensor.matmul(
                out=stak_ps[:],
                lhsT=v2R[:, b, c, :],
                rhs=st4R[:, b, :, c],
                start=(c == 0), stop=(c == TC - 1),
            )
        stak_b = small.tile([2, J], f32, tag=f"stak{b}")
        nc.scalar.copy(stak_b[:], stak_ps[:])
        staks.append(stak_b)

    # ---------------- per-sample gather + tot ------------------------------
    gsel = small.tile([J, BL], fp16, tag="gsel")
    nc.gpsimd.memset(gsel[:], 0.0)
    totAB_ps = ps_tot.tile([128, 2 * BL * H], f32, tag="totAB")
    totA_ps = totAB_ps[:, 0 : BL * H]
    totB_ps = totAB_ps[0 : S - 128, BL * H : 2 * BL * H]
    for b in range(BL):
        pst_ps = ps_sm.tile([J, 2], f32, tag="sm")
        nc.tensor.transpose(pst_ps[:], staks[b][:], id4[0:2, 0:2])
        idx_b = small.tile([J, 1], i32, tag=f"idx{b}")
        nc.scalar.copy(idx_b[:], pst_ps[:, 0:1])
        nc.scalar.copy(gselb[b][:, b : b + 1], pst_ps[:, 1:2])
        gat_b = gatp.tile([J, HS], fp16, tag="gat")
        nc.gpsimd.indirect_dma_start(
            out=gat_b[:],
            out_offset=None,
            in_=att[:],
            in_offset=bass.IndirectOffsetOnAxis(ap=idx_b[:, 0:1], axis=0),
        )
        for h in range(H):
            nc.tensor.matmul(
                out=totA_ps[:, b * H + h : b * H + h + 1],
                lhsT=gat_b[:, h * S : h * S + 128],
                rhs=gsel[:, b : b + 1],
                start=True, stop=True,
            )
        for h in range(H):
            nc.tensor.matmul(
                out=totB_ps[:, b * H + h : b * H + h + 1],
                lhsT=gat_b[:, h * S + 128 : (h + 1) * S],
                rhs=gsel[:, b : b + 1],
                start=True, stop=True,
            )

    # ---------------- transpose back + normalize ---------------------------
    totA32 = small.tile([128, BL], f32, tag="totA32")
    nc.vector.tensor_reduce(
        totA32[:].unsqueeze(2),
        totA_ps.rearrange("p (b h) -> p b h", h=H),
        axis=AX.X, op=OP.add,
    )
    totB32 = small.tile([S - 128, BL], f32, tag="totB32")
    nc.vector.tensor_reduce(
        totB32[:].unsqueeze(2),
        totB_ps.rearrange("p (b h) -> p b h", h=H),
        axis=AX.X, op=OP.add,
    )
    totA16 = small.tile([128, BL], fp16, tag="totA16")
    nc.scalar.copy(totA16[:], totA32[:])
    totB16 = small.tile([S - 128, BL], fp16, tag="totB16")
    nc.scalar.copy(totB16[:], totB32[:])
    trAB_ps = ps_tot.tile([BL, S], fp16, tag="trAB")
    nc.tensor.transpose(trAB_ps[:, 0:128], totA16[:], id16[:])
    nc.tensor.transpose(trAB_ps[:, 128:S], totB16[:], id16[0 : S - 128, 0 : S - 128])
    tot16 = small.tile([BL, S], fp16, tag="tot16")
    nc.scalar.copy(tot16[:], trAB_ps[:])

    mn = small.tile([BL, 1], f32, tag="mn")
    nc.vector.tensor_reduce(mn[:], tot16[:], axis=AX.X, op=OP.min)
    mx = small.tile([BL, 1], f32, tag="mx")
    nc.vector.tensor_reduce(mx[:], tot16[:], axis=AX.X, op=OP.max)
    rng = small.tile([BL, 1], f32, tag="rng")
    nc.gpsimd.tensor_tensor(rng[:], mx[:], mn[:], op=OP.subtract)
    rinv = small.tile([BL, 1], f32, tag="rinv")
    nc.vector.reciprocal(rinv[:], rng[:])
    out_sb = small.tile([BL, S], f32, tag="out_sb")
    nc.vector.tensor_scalar(
        out=out_sb[:], in0=tot16[:], scalar1=mn[:, 0:1], scalar2=rinv[:, 0:1],
        op0=OP.subtract, op1=OP.mult,
    )
    nc.sync.dma_start(out[:], out_sb[:])


def build_nc(path=None):
    nc = bacc.Bacc("TRN2", target_bir_lowering=False, debug=False)
    emb = nc.dram_tensor("emb", [BL, D, T], fp16, kind="ExternalInput")
    att = nc.dram_tensor("att", [BL * T, HS], fp16, kind="ExternalInput")
    fore = nc.dram_tensor("fore", [128, BL * DC], fp16, kind="ExternalInput")
    tgtT = nc.dram_tensor("tgtT", [128, NB], i32, kind="ExternalInput")
    out = nc.dram_tensor("out", [BL, S], f32, kind="ExternalOutput")
    with ExitStack() as ctx:
        tc_ = ctx.enter_context(tile.TileContext(nc))
        build_body(ctx, tc_, emb.ap(), att.ap(), fore.ap(), tgtT.ap(), out.ap())
    nc.compile()
    return nc


_NC_CACHE = {}


def get_nc(path=None):
    if "nc" not in _NC_CACHE:
        _NC_CACHE["nc"] = build_nc()
    return _NC_CACHE["nc"]


def make_in_maps(fore_rep_encoded, target_embed, align_attns, targets):
    LAYER_ID = 2
    att_l = np.transpose(np.asarray(align_attns[LAYER_ID]), (0, 2, 1, 3))  # [B,T,H,S]
    att_l = np.ascontiguousarray(att_l, dtype=np.float16)
    emb_d = np.ascontiguousarray(
        np.swapaxes(np.asarray(target_embed), 1, 2), dtype=np.float16
    )  # [B, D, T]
    fore_np = np.asarray(fore_rep_encoded, dtype=np.float16)  # [B, D]
    tgt_np = np.asarray(targets)[:, :T].astype(np.int32)      # [B, T]
    in_maps = []
    for cidx in range(NCORES):
        sl = slice(cidx * BL, (cidx + 1) * BL)
        fore_sl = fore_np[sl]                      # [BL, D]
        # fore cols [(p), (b, dc)]: col b*DC+dc = fore[b, dc*128:(dc+1)*128]
        fore_c = np.ascontiguousarray(
            fore_sl.reshape(BL, DC, 128).transpose(2, 0, 1).reshape(128, BL * DC)
        )
        tgt_sl = tgt_np[sl]                        # [BL, T]
        # tgtT [(p), (b, c)]: col b*TC+c = tgt[b, c*128:(c+1)*128]
        tgt_T = np.ascontiguousarray(
            tgt_sl.reshape(BL, TC, 128).transpose(2, 0, 1).reshape(128, NB)
        )
        in_maps.append(
            {
                "emb": np.ascontiguousarray(emb_d[sl]),
                "att": att_l[sl].reshape(BL * T, HS),
                "fore": fore_c,
                "tgtT": tgt_T,
            }
        )
    return in_maps


def kernel(fore_rep_encoded, target_embed, align_attns, targets):
    global LAST_EXEC_NS, LAST_RESULTS
    nc = get_nc()
    in_maps = make_in_maps(fore_rep_encoded, target_embed, align_attns, targets)
    trace = bool(os.environ.get("KERNEL_TRACE"))
    try:
        res = bass_utils.run_bass_kernel_spmd(
            nc, in_maps, core_ids=list(range(NCORES)), trace=trace
        )
    except ModuleNotFoundError:
        os.environ["BASS_NEVER_TRACE"] = "1"
        res = bass_utils.run_bass_kernel_spmd(
            nc, in_maps, core_ids=list(range(NCORES)), trace=False
        )
    LAST_EXEC_NS = res.exec_time_ns
    LAST_RESULTS = res
    return np.concatenate([r["out"] for r in res.results], axis=0)
